# revision 42
# baseline (speedup 1.0000x reference)
"""Trainium2 Bass kernel for BlockAutoregressiveLinear (n_blocks=128, B=32,
in=out=4096, batch=2048), 8 NeuronCores.

Math (see reference):
    Wm = exp(W)*diag_mask + W*tril_mask          (block-diag exp, strict block-lower copy)
    s[o] = sum_i Wm[o,i]^2
    g[o] = exp(W_log_scale[o] - 0.5*ln s[o])
    y = x @ (g[:,None]*Wm).T + bias
    log_jac[k,a,b] = W[k*32+a, k*32+b] + W_log_scale[o] - 0.5*ln s[o]

Sharding: 4 column groups x 2 batch halves. Core c=(2g+h) owns 8 output
m-tiles (128 rows each) mg = 4s+g for slot s=0..7 and batch half h.
W is block-lower-triangular, so m-tile mg only needs k-tiles 0..mg. For an
SPMD-uniform instruction stream, slot s is padded to K_s = 4(s+1) k-tile
matmuls (pad tiles are host-packed zeros); every core executes the identical
program and all per-core variation lives in the input data.

The diagonal-block exp contribution is a separate per-slot matmul (a_exp
tile built on device with ACT Exp) against a host-packed copy of the
corresponding x rows (xm), so the stream tiles are pure copies of W.T
(masked on host by zeroing = data selection; all arithmetic is on device).

Matmuls run in float32r (fp32 truncated to FP22 in the PE) at full rate
with N=512 moving tiles. The weight scaling by g and the bias add are fused
into the PSUM->SBUF copyback on the ACT engine (per-partition scale/bias).
"""

import sys

sys.path.insert(0, "/opt/trn_rl_repo")

import numpy as np

NB = 128          # autoregressive blocks
BLK = 32          # block size
DIM = 4096
BATCH = 2048
NSLOT = 8         # m-tile slots per core
NGRP = 4          # column groups
NT = 2            # n-tiles per batch half
NTW = 512         # n-tile width
KT = 32           # k tiles
NWC = 8           # W stream chunks (4 k-tiles each)
NXC = 4           # x chunks per n-tile (8 k-tiles each)

# k-major stream of (kt, slot) jobs; slot s is active for kt < 4(s+1)
STREAM = [(kt, s) for kt in range(KT) for s in range(NSLOT) if kt < 4 * (s + 1)]
T_IDX = {j: t for t, j in enumerate(STREAM)}
NSTREAM = len(STREAM)  # 144


def _kt_range(kt):
    """[t0, t1) tile-index range of stream tiles belonging to k-tile kt."""
    t0 = T_IDX[(kt, kt // 4)]
    return t0, t0 + NSLOT - kt // 4


def _wc_range(c):
    """[t0, t1) tile-index range of W chunk c (k-tiles 4c..4c+3)."""
    return _kt_range(4 * c)[0], _kt_range(4 * c + 3)[1]


def _build_program():
    import concourse.tile as tile
    from concourse import bacc, mybir

    F32 = mybir.dt.float32
    F32R = mybir.dt.float32r
    AF = mybir.ActivationFunctionType

    nc = bacc.Bacc("TRN2", target_bir_lowering=False, debug=False, num_devices=8)

    ws = nc.dram_tensor("ws", [128, NSTREAM * 128], F32R, kind="ExternalInput")
    # x, host-packed per-core in SBUF layout: [p, nt, xchunk, ktile-in-chunk, n]
    xt = nc.dram_tensor("xt", [128, NT, NXC, 8, NTW], F32R, kind="ExternalInput")
    xm = nc.dram_tensor("xm", [128, NT, NSLOT, NTW], F32R, kind="ExternalInput")
    dgt = nc.dram_tensor("dgt", [128, NSLOT, BLK], F32, kind="ExternalInput")
    dgr = nc.dram_tensor("dgr", [128, NSLOT, BLK], F32, kind="ExternalInput")
    b8 = nc.dram_tensor("b8", [128, NSLOT], F32, kind="ExternalInput")
    ls8 = nc.dram_tensor("ls8", [128, NSLOT], F32, kind="ExternalInput")
    on1 = nc.dram_tensor("on1", [128, 1], F32R, kind="ExternalInput")
    yt = nc.dram_tensor("yt", [1024, 1024], F32, kind="ExternalOutput")
    lj = nc.dram_tensor("lj", [128, NSLOT, BLK], F32, kind="ExternalOutput")
    srt = nc.dram_tensor("srt", [1024], F32)  # s roundtrip scratch

    with tile.TileContext(nc) as tc:
        with (
            tc.tile_pool(name="wp", bufs=1) as wp,
            tc.tile_pool(name="xp", bufs=5) as xp,
            tc.tile_pool(name="xmp", bufs=2) as xmp,
            tc.tile_pool(name="yp", bufs=4) as yp,
            tc.tile_pool(name="smallp", bufs=1) as smallp,
            tc.tile_pool(name="psy", bufs=6, space="PSUM") as pyp,
        ):
            xc_sb = {}

            def emit_xc(nt, c, eng=None):
                xc = xp.tile([128, 8, NTW], F32R, tag="xc", name=f"xc{nt}_{c}")
                (eng or nc.sync).dma_start(out=xc, in_=xt[:, nt, c, :, :])
                xc_sb[(nt, c)] = xc

            # ---- small constants (gpsimd queue; keeps sync free for bulk) ----
            ones = smallp.tile([128, 1], F32R)
            nc.gpsimd.dma_start(out=ones, in_=on1[:, :])
            dgt_sb = smallp.tile([128, NSLOT, BLK], F32)
            nc.gpsimd.dma_start(out=dgt_sb, in_=dgt[:, :, :])
            dgr_sb = smallp.tile([128, NSLOT, BLK], F32)
            nc.gpsimd.dma_start(out=dgr_sb, in_=dgr[:, :, :])
            b8_sb = smallp.tile([128, NSLOT], F32)
            nc.gpsimd.dma_start(out=b8_sb, in_=b8[:, :])
            ls8_sb = smallp.tile([128, NSLOT], F32)
            nc.gpsimd.dma_start(out=ls8_sb, in_=ls8[:, :])

            # compact exp of diagonal blocks: a_cmp[p, s, a] = exp(dgt[p, s, a])
            # in one ACT op, then scatter the 32x32 sub-blocks onto the
            # block-diagonal of a_exp with 4 SBUF->SBUF DMAs (zero elsewhere).
            a_cmp = smallp.tile([128, NSLOT, BLK], F32R)
            nc.scalar.activation(
                a_cmp.rearrange("p s a -> p (s a)"),
                dgt_sb.rearrange("p s a -> p (s a)"),
                AF.Exp,
            )
            zmem = smallp.tile([128, NSLOT * 128], F32)
            nc.vector.memset(zmem, 0.0)
            a_exp = smallp.tile([128, NSLOT, 128], F32R)
            nc.vector.tensor_copy(a_exp.rearrange("p s f -> p (s f)"), zmem)
            for j in range(4):
                pr = slice(32 * j, 32 * j + 32)
                nc.gpsimd.dma_start(out=a_exp[pr, :, 32 * j : 32 * j + 32], in_=a_cmp[pr, :, :])

            w_chunk = []

            def w_tile(kt, s):
                t = T_IDX[(kt, s)]
                c = kt // 4
                t0, _ = _wc_range(c)
                return w_chunk[c][:, (t - t0) * 128 : (t - t0 + 1) * 128]

            def emit_matmuls(s, nt, out_sb, copy_raw):
                """Accumulate slot s, n-tile nt into PSUM, then copy to out_sb.

                copy_raw=True: plain PSUM->SBUF copy (g not available yet);
                the g*psum+bias affine is applied later in place.
                copy_raw=False: fused g*psum+bias copyback.
                """
                xms = xmp.tile([128, NTW], F32R, tag="xm", name=f"xm{nt}_{s}")
                nc.gpsimd.dma_start(out=xms, in_=xm[:, nt, s, :])
                ps_y = pyp.tile([128, NTW], F32, tag="psy")
                for kt in range(4 * (s + 1)):
                    nc.tensor.matmul(
                        ps_y,
                        w_tile(kt, s),
                        xc_sb[(nt, kt // 8)][:, kt % 8, :],
                        start=(kt == 0),
                        stop=False,
                    )
                # diag-block exp contribution (block-diagonal lhsT, full K)
                nc.tensor.matmul(
                    ps_y, a_exp[:, s, :], xms, start=False, stop=True
                )
                if copy_raw:
                    nc.scalar.copy(out_sb, ps_y)
                else:
                    nc.scalar.activation(
                        out_sb, ps_y, AF.Identity,
                        bias=b8_sb[:, s : s + 1], scale=g_sb[:, s : s + 1],
                    )

            g_sb = smallp.tile([128, NSLOT], F32)
            ljb = smallp.tile([128, NSLOT], F32)
            # staging for nt=0 results (copied raw before g is known)
            stage = [
                smallp.tile([128, NTW], F32, tag=f"st{s}", name=f"stage{s}")
                for s in range(NSLOT)
            ]

            # ---- interleaved: W chunk DMA -> norm jobs -> main slot (nt=0) ----
            # DMA triggers are emitted in consumption order (wc0, xc0, wc1,
            # wc2, xc1, ...) so early slots' data arrives first; main slot
            # emission lags the chunk loop by 2 so the PE never parks on the
            # a_exp-dependent tail matmul while W/x are still streaming.
            with (
                tc.tile_pool(name="psn", bufs=1, space="PSUM") as pnp,
                tc.tile_pool(name="sqp", bufs=2) as sqp,
            ):
                psn = pnp.tile([1, 1024], F32)

                # W chunk tiles created up front; chunks 0-3 DMA'd immediately
                # (first wave), chunks 4-7 triggered from the ACT engine after
                # early slots' copybacks so DMA issue paces with compute.
                for c in range(NWC):
                    t0c, t1c = _wc_range(c)
                    wc = wp.tile([128, (t1c - t0c) * 128], F32R, tag=f"wc{c}", name=f"wct{c}")
                    w_chunk.append(wc)

                def trigger_wc(c, eng):
                    t0c, _ = _wc_range(c)
                    t1c = _wc_range(c)[1]
                    eng.dma_start(out=w_chunk[c], in_=ws[:, t0c * 128 : t1c * 128])

                emit_xc(0, 0)
                trigger_wc(0, nc.sync)
                trigger_wc(1, nc.sync)
                emit_xc(0, 1)
                trigger_wc(2, nc.sync)
                trigger_wc(3, nc.sync)

                # DMA triggers injected after slot s's (nt=0) copyback, keyed
                # by slot index: the ACT engine reaches them only once that
                # slot's matmuls are done, so late chunks don't steal HBM
                # bandwidth from early ones.
                paced = {
                    0: [lambda: trigger_wc(4, nc.scalar)],
                    1: [lambda: trigger_wc(5, nc.scalar),
                        lambda: emit_xc(0, 2, nc.scalar)],
                    2: [lambda: trigger_wc(6, nc.scalar)],
                    3: [lambda: trigger_wc(7, nc.scalar),
                        lambda: emit_xc(0, 3, nc.scalar)],
                    4: [lambda: emit_xc(1, 0, nc.scalar)],
                    5: [lambda: emit_xc(1, 1, nc.scalar)],
                    6: [lambda: emit_xc(1, 2, nc.scalar)],
                    7: [lambda: emit_xc(1, 3, nc.scalar)],
                }

                for c in range(NWC):
                    c0 = _wc_range(c)[0]
                    for kt in range(4 * c, 4 * c + 4):
                        t0, t1 = _kt_range(kt)
                        nact = t1 - t0
                        w_slice = w_chunk[c][:, (t0 - c0) * 128 : (t1 - c0) * 128]
                        smin = c * 128
                        off = 0
                        while off < nact * 128:
                            n = min(512, nact * 128 - off)
                            sq = sqp.tile([128, 512], F32R, tag="sq")
                            nc.vector.tensor_mul(
                                sq[:, :n],
                                w_slice[:, off : off + n],
                                w_slice[:, off : off + n],
                            )
                            nc.tensor.matmul(
                                psn[0:1, smin + off : smin + off + n],
                                ones,
                                sq[:, :n],
                                start=(kt == 0),
                                stop=False,
                            )
                            off += n
                    # lag main-slot emission 2 chunks behind the DMA stream
                    if c >= 2:
                        emit_matmuls(c - 2, 0, stage[c - 2], copy_raw=True)
                        for fn in paced.get(c - 2, []):
                            fn()
                for s in range(NSLOT - 2, NSLOT):
                    emit_matmuls(s, 0, stage[s], copy_raw=True)
                    for fn in paced.get(s, []):
                        fn()
                # diag-block contribution to the norms (block-diagonal squares)
                ae_flat = a_exp.rearrange("p s f -> p (s f)")
                for half in range(2):
                    a_sq = sqp.tile([128, 512], F32R, tag="sq", name=f"asq{half}")
                    nc.vector.tensor_mul(
                        a_sq, ae_flat[:, half * 512 : (half + 1) * 512],
                        ae_flat[:, half * 512 : (half + 1) * 512],
                    )
                    nc.tensor.matmul(
                        psn[0:1, half * 512 : (half + 1) * 512], ones, a_sq,
                        start=False, stop=(half == 1),
                    )
                s_row = smallp.tile([1, 1024], F32)
                nc.scalar.copy(s_row, psn[0:1, :])

            # s roundtrip through DRAM to get per-partition layout [128, 8]
            nc.gpsimd.dma_start(out=srt[:].rearrange("(a n) -> a n", a=1), in_=s_row[0:1, :])
            s_col = smallp.tile([128, NSLOT], F32)
            nc.gpsimd.dma_start(out=s_col, in_=srt[:].rearrange("(s p) -> p s", p=128))

            # g = exp(ls - 0.5 ln s);  ljb = ls - 0.5 ln s
            t_ln = smallp.tile([128, NSLOT], F32)
            nc.scalar.activation(t_ln, s_col, AF.Ln)
            for s in range(NSLOT):
                nc.scalar.activation(
                    ljb[:, s : s + 1], t_ln[:, s : s + 1], AF.Identity,
                    bias=ls8_sb[:, s : s + 1], scale=-0.5,
                )
            nc.scalar.activation(g_sb, ljb, AF.Exp)

            # log-jacobian: lj = W_diag + (ls - 0.5 ln s)
            lj_sb = smallp.tile([128, NSLOT, BLK], F32)
            for s in range(NSLOT):
                nc.scalar.activation(
                    lj_sb[:, s, :], dgr_sb[:, s, :], AF.Identity,
                    bias=ljb[:, s : s + 1], scale=1.0,
                )
            nc.gpsimd.dma_start(out=lj[:, :, :], in_=lj_sb)

            # nt=0: apply y = g*acc + bias in place, then store
            for s in range(NSLOT):
                nc.scalar.activation(
                    stage[s], stage[s], AF.Identity,
                    bias=b8_sb[:, s : s + 1], scale=g_sb[:, s : s + 1],
                )
                nc.scalar.dma_start(
                    out=yt[s * 128 : (s + 1) * 128, 0:NTW], in_=stage[s]
                )

            # ---- second batch half (nt=1; x chunks already paced in) ----
            for s in range(NSLOT):
                y_sb = yp.tile([128, NTW], F32, tag="y")
                emit_matmuls(s, 1, y_sb, copy_raw=False)
                nc.scalar.dma_start(
                    out=yt[s * 128 : (s + 1) * 128, NTW : 2 * NTW], in_=y_sb
                )
    nc.finalize()
    return nc


def _pack_inputs(x, W, bias, W_log_scale):
    """Host-side data layout: slice/transpose/zero-select only (no math)."""
    f32 = np.float32
    WT = np.ascontiguousarray(W.T.astype(f32, copy=False))
    xT = np.ascontiguousarray(x.T.astype(f32, copy=False))
    bias = bias.astype(f32, copy=False)
    ls = W_log_scale.astype(f32, copy=False)

    grp = {}
    for g in range(NGRP):
        ws_g = np.zeros((128, NSTREAM, 128), f32)
        for t, (kt, s) in enumerate(STREAM):
            mg = 4 * s + g
            if kt < mg:
                ws_g[:, t, :] = WT[kt * 128 : (kt + 1) * 128, mg * 128 : (mg + 1) * 128]
            elif kt == mg:
                blk = WT[kt * 128 : (kt + 1) * 128, mg * 128 : (mg + 1) * 128].copy()
                for jp in range(4):  # keep only sub-blocks strictly above the diagonal
                    blk[32 * jp : 32 * jp + 32, : 32 * (jp + 1)] = 0.0
                ws_g[:, t, :] = blk
            # kt > mg: zero pad
        dgt = np.empty((128, NSLOT, BLK), f32)
        dgr = np.empty((128, NSLOT, BLK), f32)
        for s in range(NSLOT):
            mg = 4 * s + g
            blk = W[mg * 128 : (mg + 1) * 128, mg * 128 : (mg + 1) * 128]
            for j in range(4):
                sub = blk[32 * j : 32 * j + 32, 32 * j : 32 * j + 32]  # [a, b]
                dgt[32 * j : 32 * j + 32, s, :] = sub.T
                dgr[32 * j : 32 * j + 32, s, :] = sub
        b8 = np.stack(
            [bias[(4 * s + g) * 128 : (4 * s + g + 1) * 128] for s in range(NSLOT)], 1
        )
        ls8 = np.stack(
            [ls[(4 * s + g) * 128 : (4 * s + g + 1) * 128, 0] for s in range(NSLOT)], 1
        )
        grp[g] = dict(
            ws=np.ascontiguousarray(ws_g.reshape(128, NSTREAM * 128)),
            dgt=dgt, dgr=dgr, b8=np.ascontiguousarray(b8), ls8=np.ascontiguousarray(ls8),
        )

    # x in per-core SBUF layout: xt[p, nt, c, j, n] = xT[(8c+j)*128+p, h*1024+nt*512+n]
    xt_h = {}
    xm_h = {}
    for h in range(2):
        xs = xT[:, h * 1024 : (h + 1) * 1024]               # [4096, 1024]
        v = xs.reshape(KT, 128, NT, NTW)                     # [kt, p, nt, n]
        v = v.transpose(1, 2, 0, 3)                          # [p, nt, kt, n]
        xt_h[h] = np.ascontiguousarray(v.reshape(128, NT, NXC, 8, NTW))
    in_maps = []
    for c in range(8):
        g, h = c // 2, c % 2
        xm_c = np.empty((128, NT, NSLOT, NTW), f32)
        for s in range(NSLOT):
            mg = 4 * s + g
            for nt in range(NT):
                xm_c[:, nt, s, :] = xT[
                    mg * 128 : (mg + 1) * 128,
                    h * 1024 + nt * NTW : h * 1024 + (nt + 1) * NTW,
                ]
        in_maps.append(
            {
                **grp[g],
                "xt": xt_h[h],
                "xm": xm_c,
                "on1": np.ones((128, 1), f32),
            }
        )
    return in_maps


_PROGRAM = None


def kernel(x, W, bias, W_log_scale, b_diag_mask=None, b_tril_mask=None, **_ignored):
    """Full inputs in, full outputs out. Returns (y, log_jac) like the reference.

    The diag/tril masks are the fixed block-kronecker patterns of the module
    (block-diagonal + strict block-lower-triangular); the kernel exploits that
    structure directly, so the mask tensors themselves are not transferred.
    """
    from concourse.bass_utils import run_bass_kernel_spmd

    global _PROGRAM
    if _PROGRAM is None:
        _PROGRAM = _build_program()
    nc = _PROGRAM

    in_maps = _pack_inputs(
        np.asarray(x), np.asarray(W), np.asarray(bias), np.asarray(W_log_scale)
    )
    res = run_bass_kernel_spmd(nc, in_maps, list(range(8)))

    y = np.empty((BATCH, DIM), np.float32)
    lj_full = np.empty((NB, BLK, BLK), np.float32)
    for c, r in enumerate(res.results):
        g, h = c // 2, c % 2
        ytc = r["yt"]
        for s in range(NSLOT):
            mg = 4 * s + g
            y[h * 1024 : (h + 1) * 1024, mg * 128 : (mg + 1) * 128] = (
                ytc[s * 128 : (s + 1) * 128, :].T
            )
        if h == 0:
            ljc = r["lj"]  # [128, 8, 32]
            for s in range(NSLOT):
                mg = 4 * s + g
                for j in range(4):
                    lj_full[4 * mg + j] = ljc[32 * j : 32 * j + 32, s, :]
    return (y, lj_full)


# revision 46
# speedup vs baseline: 1.3052x; 1.3052x over previous
"""Trainium2 Bass kernel for BlockAutoregressiveLinear (n_blocks=128, B=32,
in=out=4096, batch=2048), 8 NeuronCores.

Math (see reference):
    Wm = exp(W)*diag_mask + W*tril_mask          (block-diag exp, strict block-lower copy)
    s[o] = sum_i Wm[o,i]^2
    g[o] = exp(W_log_scale[o] - 0.5*ln s[o])
    y = x @ (g[:,None]*Wm).T + bias
    log_jac[k,a,b] = W[k*32+a, k*32+b] + W_log_scale[o] - 0.5*ln s[o]

Sharding: 4 column groups x 2 batch halves. Core c=(2g+h) owns 8 output
m-tiles (128 rows each) mg = 4s+g for slot s=0..7 and batch half h.
W is block-lower-triangular, so m-tile mg only needs k-tiles 0..mg. For an
SPMD-uniform instruction stream, slot s is padded to K_s = 4(s+1) k-tile
matmuls (pad tiles are host-packed zeros); every core executes the identical
program and all per-core variation lives in the input data.

The diagonal-block exp contribution is a separate per-slot matmul (a_exp
tile built on device with ACT Exp) against a host-packed copy of the
corresponding x rows (xm), so the stream tiles are pure copies of W.T
(masked on host by zeroing = data selection; all arithmetic is on device).

Matmuls run in float32r (fp32 truncated to FP22 in the PE) at full rate
with N=512 moving tiles. The weight scaling by g and the bias add are fused
into the PSUM->SBUF copyback on the ACT engine (per-partition scale/bias).
"""

import sys

sys.path.insert(0, "/opt/trn_rl_repo")

import numpy as np

NB = 128          # autoregressive blocks
BLK = 32          # block size
DIM = 4096
BATCH = 2048
NSLOT = 8         # m-tile slots per core
NGRP = 4          # column groups
NT = 2            # n-tiles per batch half
NTW = 512         # n-tile width
KT = 32           # k tiles
NWC = 8           # W stream chunks (4 k-tiles each)
NXC = 4           # x chunks per n-tile (8 k-tiles each)

# k-major stream of (kt, slot) jobs; slot s is active for kt < 4(s+1)
STREAM = [(kt, s) for kt in range(KT) for s in range(NSLOT) if kt < 4 * (s + 1)]
T_IDX = {j: t for t, j in enumerate(STREAM)}
NSTREAM = len(STREAM)  # 144


def _kt_range(kt):
    """[t0, t1) tile-index range of stream tiles belonging to k-tile kt."""
    t0 = T_IDX[(kt, kt // 4)]
    return t0, t0 + NSLOT - kt // 4


def _wc_range(c):
    """[t0, t1) tile-index range of W chunk c (k-tiles 4c..4c+3)."""
    return _kt_range(4 * c)[0], _kt_range(4 * c + 3)[1]


def _build_program():
    import concourse.tile as tile
    from concourse import bacc, mybir

    F32 = mybir.dt.float32
    F32R = mybir.dt.float32r
    AF = mybir.ActivationFunctionType

    nc = bacc.Bacc("TRN2", target_bir_lowering=False, debug=False, num_devices=8)

    ws = nc.dram_tensor("ws", [128, NSTREAM * 128], F32R, kind="ExternalInput")
    # x, host-packed per-core in SBUF layout: [p, nt, xchunk, ktile-in-chunk, n]
    xt = nc.dram_tensor("xt", [128, NT, NXC, 8, NTW], F32R, kind="ExternalInput")
    xm = nc.dram_tensor("xm", [128, NT, NSLOT, NTW], F32R, kind="ExternalInput")
    dgt = nc.dram_tensor("dgt", [128, NSLOT, BLK], F32, kind="ExternalInput")
    dgr = nc.dram_tensor("dgr", [128, NSLOT, BLK], F32, kind="ExternalInput")
    b8 = nc.dram_tensor("b8", [128, NSLOT], F32, kind="ExternalInput")
    ls8 = nc.dram_tensor("ls8", [128, NSLOT], F32, kind="ExternalInput")
    on1 = nc.dram_tensor("on1", [128, 1], F32R, kind="ExternalInput")
    yt = nc.dram_tensor("yt", [1024, 1024], F32, kind="ExternalOutput")
    lj = nc.dram_tensor("lj", [128, NSLOT, BLK], F32, kind="ExternalOutput")
    srt = nc.dram_tensor("srt", [1024], F32)  # s roundtrip scratch

    with tile.TileContext(nc) as tc:
        with (
            tc.tile_pool(name="wp", bufs=1) as wp,
            tc.tile_pool(name="xp", bufs=5) as xp,
            tc.tile_pool(name="xmp", bufs=2) as xmp,
            tc.tile_pool(name="yp", bufs=4) as yp,
            tc.tile_pool(name="smallp", bufs=1) as smallp,
            tc.tile_pool(name="psy", bufs=6, space="PSUM") as pyp,
        ):
            xc_sb = {}

            def emit_xc(nt, c, eng=None):
                # split into 4 sub-DMAs so the transfer spreads across DGE
                # queues (more SDMA engines in parallel)
                xc = xp.tile([128, 8, NTW], F32R, tag="xc", name=f"xc{nt}_{c}")
                e = eng or nc.sync
                for q in range(4):
                    e.dma_start(
                        out=xc[:, 2 * q : 2 * q + 2, :],
                        in_=xt[:, nt, c, 2 * q : 2 * q + 2, :],
                    )
                xc_sb[(nt, c)] = xc

            # ---- small constants (gpsimd queue; keeps sync free for bulk) ----
            ones = smallp.tile([128, 1], F32R)
            nc.gpsimd.dma_start(out=ones, in_=on1[:, :])
            dgt_sb = smallp.tile([128, NSLOT, BLK], F32)
            nc.gpsimd.dma_start(out=dgt_sb, in_=dgt[:, :, :])
            dgr_sb = smallp.tile([128, NSLOT, BLK], F32)
            nc.gpsimd.dma_start(out=dgr_sb, in_=dgr[:, :, :])
            b8_sb = smallp.tile([128, NSLOT], F32)
            nc.gpsimd.dma_start(out=b8_sb, in_=b8[:, :])
            ls8_sb = smallp.tile([128, NSLOT], F32)
            nc.gpsimd.dma_start(out=ls8_sb, in_=ls8[:, :])

            # compact exp of diagonal blocks: a_cmp[p, s, a] = exp(dgt[p, s, a])
            # in one ACT op, then scatter the 32x32 sub-blocks onto the
            # block-diagonal of a_exp with 4 SBUF->SBUF DMAs (zero elsewhere).
            a_cmp = smallp.tile([128, NSLOT, BLK], F32R)
            nc.scalar.activation(
                a_cmp.rearrange("p s a -> p (s a)"),
                dgt_sb.rearrange("p s a -> p (s a)"),
                AF.Exp,
            )
            zmem = smallp.tile([128, NSLOT * 128], F32)
            nc.vector.memset(zmem, 0.0)
            a_exp = smallp.tile([128, NSLOT, 128], F32R)
            nc.vector.tensor_copy(a_exp.rearrange("p s f -> p (s f)"), zmem)
            for j in range(4):
                pr = slice(32 * j, 32 * j + 32)
                nc.gpsimd.dma_start(out=a_exp[pr, :, 32 * j : 32 * j + 32], in_=a_cmp[pr, :, :])

            w_chunk = []

            def w_tile(kt, s):
                t = T_IDX[(kt, s)]
                c = kt // 4
                t0, _ = _wc_range(c)
                return w_chunk[c][:, (t - t0) * 128 : (t - t0 + 1) * 128]

            def emit_matmuls(s, nt, out_sb, copy_raw):
                """Accumulate slot s, n-tile nt into PSUM, then copy to out_sb.

                copy_raw=True: plain PSUM->SBUF copy (g not available yet);
                the g*psum+bias affine is applied later in place.
                copy_raw=False: fused g*psum+bias copyback.
                """
                xms = xmp.tile([128, NTW], F32R, tag="xm", name=f"xm{nt}_{s}")
                nc.gpsimd.dma_start(out=xms, in_=xm[:, nt, s, :])
                ps_y = pyp.tile([128, NTW], F32, tag="psy")
                for kt in range(4 * (s + 1)):
                    nc.tensor.matmul(
                        ps_y,
                        w_tile(kt, s),
                        xc_sb[(nt, kt // 8)][:, kt % 8, :],
                        start=(kt == 0),
                        stop=False,
                    )
                # diag-block exp contribution (block-diagonal lhsT, full K)
                nc.tensor.matmul(
                    ps_y, a_exp[:, s, :], xms, start=False, stop=True
                )
                if copy_raw:
                    nc.scalar.copy(out_sb, ps_y)
                else:
                    nc.scalar.activation(
                        out_sb, ps_y, AF.Identity,
                        bias=b8_sb[:, s : s + 1], scale=g_sb[:, s : s + 1],
                    )

            g_sb = smallp.tile([128, NSLOT], F32)
            ljb = smallp.tile([128, NSLOT], F32)
            # staging for nt=0 results (copied raw before g is known)
            stage = [
                smallp.tile([128, NTW], F32, tag=f"st{s}", name=f"stage{s}")
                for s in range(NSLOT)
            ]

            # ---- interleaved: W chunk DMA -> norm jobs -> main slot (nt=0) ----
            # DMA triggers are emitted in consumption order (wc0, xc0, wc1,
            # wc2, xc1, ...) so early slots' data arrives first; main slot
            # emission lags the chunk loop by 2 so the PE never parks on the
            # a_exp-dependent tail matmul while W/x are still streaming.
            with (
                tc.tile_pool(name="psn", bufs=1, space="PSUM") as pnp,
                tc.tile_pool(name="sqp", bufs=2) as sqp,
            ):
                psn = pnp.tile([1, 1024], F32)

                # W chunk tiles created up front; chunks 0-3 DMA'd immediately
                # (first wave), chunks 4-7 triggered from the ACT engine after
                # early slots' copybacks so DMA issue paces with compute.
                for c in range(NWC):
                    t0c, t1c = _wc_range(c)
                    wc = wp.tile([128, (t1c - t0c) * 128], F32R, tag=f"wc{c}", name=f"wct{c}")
                    w_chunk.append(wc)

                def trigger_wc(c, eng):
                    t0c, t1c = _wc_range(c)
                    # split in 2 sub-DMAs for queue spread
                    mid = (t0c + t1c) // 2
                    eng.dma_start(
                        out=w_chunk[c][:, : (mid - t0c) * 128],
                        in_=ws[:, t0c * 128 : mid * 128],
                    )
                    eng.dma_start(
                        out=w_chunk[c][:, (mid - t0c) * 128 :],
                        in_=ws[:, mid * 128 : t1c * 128],
                    )

                # everything up front, consumption-ordered; the DGE round-robin
                # shares bandwidth, order mostly sets start order
                trigger_wc(0, nc.sync)
                emit_xc(0, 0)
                trigger_wc(1, nc.sync)
                emit_xc(0, 1)
                trigger_wc(2, nc.sync)
                trigger_wc(3, nc.sync)
                emit_xc(0, 2)
                trigger_wc(4, nc.sync)
                trigger_wc(5, nc.sync)
                emit_xc(0, 3)
                trigger_wc(6, nc.sync)
                trigger_wc(7, nc.sync)

                for c in range(NWC):
                    c0 = _wc_range(c)[0]
                    for kt in range(4 * c, 4 * c + 4):
                        t0, t1 = _kt_range(kt)
                        nact = t1 - t0
                        w_slice = w_chunk[c][:, (t0 - c0) * 128 : (t1 - c0) * 128]
                        smin = c * 128
                        off = 0
                        while off < nact * 128:
                            n = min(512, nact * 128 - off)
                            sq = sqp.tile([128, 512], F32R, tag="sq")
                            nc.vector.tensor_mul(
                                sq[:, :n],
                                w_slice[:, off : off + n],
                                w_slice[:, off : off + n],
                            )
                            nc.tensor.matmul(
                                psn[0:1, smin + off : smin + off + n],
                                ones,
                                sq[:, :n],
                                start=(kt == 0),
                                stop=False,
                            )
                            off += n
                    # lag main-slot emission 2 chunks behind the DMA stream
                    if c >= 2:
                        emit_matmuls(c - 2, 0, stage[c - 2], copy_raw=True)
                for s in range(NSLOT - 2, NSLOT):
                    emit_matmuls(s, 0, stage[s], copy_raw=True)
                # diag-block contribution to the norms (block-diagonal squares)
                ae_flat = a_exp.rearrange("p s f -> p (s f)")
                for half in range(2):
                    a_sq = sqp.tile([128, 512], F32R, tag="sq", name=f"asq{half}")
                    nc.vector.tensor_mul(
                        a_sq, ae_flat[:, half * 512 : (half + 1) * 512],
                        ae_flat[:, half * 512 : (half + 1) * 512],
                    )
                    nc.tensor.matmul(
                        psn[0:1, half * 512 : (half + 1) * 512], ones, a_sq,
                        start=False, stop=(half == 1),
                    )
                s_row = smallp.tile([1, 1024], F32)
                nc.scalar.copy(s_row, psn[0:1, :])

            # s roundtrip through DRAM to get per-partition layout [128, 8]
            nc.gpsimd.dma_start(out=srt[:].rearrange("(a n) -> a n", a=1), in_=s_row[0:1, :])
            s_col = smallp.tile([128, NSLOT], F32)
            nc.gpsimd.dma_start(out=s_col, in_=srt[:].rearrange("(s p) -> p s", p=128))

            # g = exp(ls - 0.5 ln s);  ljb = ls - 0.5 ln s
            t_ln = smallp.tile([128, NSLOT], F32)
            nc.scalar.activation(t_ln, s_col, AF.Ln)
            for s in range(NSLOT):
                nc.scalar.activation(
                    ljb[:, s : s + 1], t_ln[:, s : s + 1], AF.Identity,
                    bias=ls8_sb[:, s : s + 1], scale=-0.5,
                )
            nc.scalar.activation(g_sb, ljb, AF.Exp)

            # log-jacobian: lj = W_diag + (ls - 0.5 ln s)
            lj_sb = smallp.tile([128, NSLOT, BLK], F32)
            for s in range(NSLOT):
                nc.scalar.activation(
                    lj_sb[:, s, :], dgr_sb[:, s, :], AF.Identity,
                    bias=ljb[:, s : s + 1], scale=1.0,
                )
            nc.gpsimd.dma_start(out=lj[:, :, :], in_=lj_sb)

            # nt=0: apply y = g*acc + bias in place, then store
            for s in range(NSLOT):
                nc.scalar.activation(
                    stage[s], stage[s], AF.Identity,
                    bias=b8_sb[:, s : s + 1], scale=g_sb[:, s : s + 1],
                )
                nc.scalar.dma_start(
                    out=yt[s * 128 : (s + 1) * 128, 0:NTW], in_=stage[s]
                )

            # ---- second batch half (nt=1) ----
            for c in range(NXC):
                emit_xc(1, c)
            for s in range(NSLOT):
                y_sb = yp.tile([128, NTW], F32, tag="y")
                emit_matmuls(s, 1, y_sb, copy_raw=False)
                nc.scalar.dma_start(
                    out=yt[s * 128 : (s + 1) * 128, NTW : 2 * NTW], in_=y_sb
                )
    nc.finalize()
    return nc


def _pack_inputs(x, W, bias, W_log_scale):
    """Host-side data layout: slice/transpose/zero-select only (no math)."""
    f32 = np.float32
    WT = np.ascontiguousarray(W.T.astype(f32, copy=False))
    xT = np.ascontiguousarray(x.T.astype(f32, copy=False))
    bias = bias.astype(f32, copy=False)
    ls = W_log_scale.astype(f32, copy=False)

    grp = {}
    for g in range(NGRP):
        ws_g = np.zeros((128, NSTREAM, 128), f32)
        for t, (kt, s) in enumerate(STREAM):
            mg = 4 * s + g
            if kt < mg:
                ws_g[:, t, :] = WT[kt * 128 : (kt + 1) * 128, mg * 128 : (mg + 1) * 128]
            elif kt == mg:
                blk = WT[kt * 128 : (kt + 1) * 128, mg * 128 : (mg + 1) * 128].copy()
                for jp in range(4):  # keep only sub-blocks strictly above the diagonal
                    blk[32 * jp : 32 * jp + 32, : 32 * (jp + 1)] = 0.0
                ws_g[:, t, :] = blk
            # kt > mg: zero pad
        dgt = np.empty((128, NSLOT, BLK), f32)
        dgr = np.empty((128, NSLOT, BLK), f32)
        for s in range(NSLOT):
            mg = 4 * s + g
            blk = W[mg * 128 : (mg + 1) * 128, mg * 128 : (mg + 1) * 128]
            for j in range(4):
                sub = blk[32 * j : 32 * j + 32, 32 * j : 32 * j + 32]  # [a, b]
                dgt[32 * j : 32 * j + 32, s, :] = sub.T
                dgr[32 * j : 32 * j + 32, s, :] = sub
        b8 = np.stack(
            [bias[(4 * s + g) * 128 : (4 * s + g + 1) * 128] for s in range(NSLOT)], 1
        )
        ls8 = np.stack(
            [ls[(4 * s + g) * 128 : (4 * s + g + 1) * 128, 0] for s in range(NSLOT)], 1
        )
        grp[g] = dict(
            ws=np.ascontiguousarray(ws_g.reshape(128, NSTREAM * 128)),
            dgt=dgt, dgr=dgr, b8=np.ascontiguousarray(b8), ls8=np.ascontiguousarray(ls8),
        )

    # x in per-core SBUF layout: xt[p, nt, c, j, n] = xT[(8c+j)*128+p, h*1024+nt*512+n]
    xt_h = {}
    xm_h = {}
    for h in range(2):
        xs = xT[:, h * 1024 : (h + 1) * 1024]               # [4096, 1024]
        v = xs.reshape(KT, 128, NT, NTW)                     # [kt, p, nt, n]
        v = v.transpose(1, 2, 0, 3)                          # [p, nt, kt, n]
        xt_h[h] = np.ascontiguousarray(v.reshape(128, NT, NXC, 8, NTW))
    in_maps = []
    for c in range(8):
        g, h = c // 2, c % 2
        xm_c = np.empty((128, NT, NSLOT, NTW), f32)
        for s in range(NSLOT):
            mg = 4 * s + g
            for nt in range(NT):
                xm_c[:, nt, s, :] = xT[
                    mg * 128 : (mg + 1) * 128,
                    h * 1024 + nt * NTW : h * 1024 + (nt + 1) * NTW,
                ]
        in_maps.append(
            {
                **grp[g],
                "xt": xt_h[h],
                "xm": xm_c,
                "on1": np.ones((128, 1), f32),
            }
        )
    return in_maps


_PROGRAM = None


def kernel(x, W, bias, W_log_scale, b_diag_mask=None, b_tril_mask=None, **_ignored):
    """Full inputs in, full outputs out. Returns (y, log_jac) like the reference.

    The diag/tril masks are the fixed block-kronecker patterns of the module
    (block-diagonal + strict block-lower-triangular); the kernel exploits that
    structure directly, so the mask tensors themselves are not transferred.
    """
    from concourse.bass_utils import run_bass_kernel_spmd

    global _PROGRAM
    if _PROGRAM is None:
        _PROGRAM = _build_program()
    nc = _PROGRAM

    in_maps = _pack_inputs(
        np.asarray(x), np.asarray(W), np.asarray(bias), np.asarray(W_log_scale)
    )
    res = run_bass_kernel_spmd(nc, in_maps, list(range(8)))

    y = np.empty((BATCH, DIM), np.float32)
    lj_full = np.empty((NB, BLK, BLK), np.float32)
    for c, r in enumerate(res.results):
        g, h = c // 2, c % 2
        ytc = r["yt"]
        for s in range(NSLOT):
            mg = 4 * s + g
            y[h * 1024 : (h + 1) * 1024, mg * 128 : (mg + 1) * 128] = (
                ytc[s * 128 : (s + 1) * 128, :].T
            )
        if h == 0:
            ljc = r["lj"]  # [128, 8, 32]
            for s in range(NSLOT):
                mg = 4 * s + g
                for j in range(4):
                    lj_full[4 * mg + j] = ljc[32 * j : 32 * j + 32, s, :]
    return (y, lj_full)


# revision 49
# speedup vs baseline: 1.6484x; 1.2630x over previous
"""Trainium2 Bass kernel for BlockAutoregressiveLinear (n_blocks=128, B=32,
in=out=4096, batch=2048), 8 NeuronCores.

Math (see reference):
    Wm = exp(W)*diag_mask + W*tril_mask          (block-diag exp, strict block-lower copy)
    s[o] = sum_i Wm[o,i]^2
    g[o] = exp(W_log_scale[o] - 0.5*ln s[o])
    y = x @ (g[:,None]*Wm).T + bias
    log_jac[k,a,b] = W[k*32+a, k*32+b] + W_log_scale[o] - 0.5*ln s[o]

Sharding: 4 column groups x 2 batch halves. Core c=(2g+h) owns 8 output
m-tiles (128 rows each) mg = 4s+g for slot s=0..7 and batch half h.
W is block-lower-triangular, so m-tile mg only needs k-tiles 0..mg. For an
SPMD-uniform instruction stream, slot s is padded to K_s = 4(s+1) k-tile
matmuls (pad tiles are host-packed zeros); every core executes the identical
program and all per-core variation lives in the input data.

The diagonal-block exp contribution is a separate per-slot matmul (a_exp
tile built on device with ACT Exp) against a host-packed copy of the
corresponding x rows (xm), so the stream tiles are pure copies of W.T
(masked on host by zeroing = data selection; all arithmetic is on device).

Main matmuls run in fp16 (the weight-normalization cancels most of the
W-quantization error; measured end-to-end rel err ~3e-4, same class as
fp32r) with N=512 moving tiles; the row-norm reduction runs in f32r.
The weight scaling by g and the bias add are fused into the PSUM->SBUF
copyback on the ACT engine (per-partition scale/bias).
"""

import sys

sys.path.insert(0, "/opt/trn_rl_repo")

import numpy as np

NB = 128          # autoregressive blocks
BLK = 32          # block size
DIM = 4096
BATCH = 2048
NSLOT = 8         # m-tile slots per core
NGRP = 4          # column groups
NT = 2            # n-tiles per batch half
NTW = 512         # n-tile width
KT = 32           # k tiles
NWC = 8           # W stream chunks (4 k-tiles each)
NXC = 4           # x chunks per n-tile (8 k-tiles each)

# k-major stream of (kt, slot) jobs; slot s is active for kt < 4(s+1)
STREAM = [(kt, s) for kt in range(KT) for s in range(NSLOT) if kt < 4 * (s + 1)]
T_IDX = {j: t for t, j in enumerate(STREAM)}
NSTREAM = len(STREAM)  # 144


def _kt_range(kt):
    """[t0, t1) tile-index range of stream tiles belonging to k-tile kt."""
    t0 = T_IDX[(kt, kt // 4)]
    return t0, t0 + NSLOT - kt // 4


def _wc_range(c):
    """[t0, t1) tile-index range of W chunk c (k-tiles 4c..4c+3)."""
    return _kt_range(4 * c)[0], _kt_range(4 * c + 3)[1]


def _build_program():
    import concourse.tile as tile
    from concourse import bacc, mybir

    F32 = mybir.dt.float32
    F32R = mybir.dt.float32r
    F16 = mybir.dt.float16
    AF = mybir.ActivationFunctionType

    nc = bacc.Bacc("TRN2", target_bir_lowering=False, debug=False, num_devices=8)

    ws = nc.dram_tensor("ws", [128, NSTREAM * 128], F16, kind="ExternalInput")
    # x, host-packed per-core in SBUF layout: [p, nt, xchunk, ktile-in-chunk, n]
    xt = nc.dram_tensor("xt", [128, NT, NXC, 8, NTW], F16, kind="ExternalInput")
    xm = nc.dram_tensor("xm", [128, NT, NSLOT, NTW], F16, kind="ExternalInput")
    dgt = nc.dram_tensor("dgt", [128, NSLOT, BLK], F32, kind="ExternalInput")
    dgr = nc.dram_tensor("dgr", [128, NSLOT, BLK], F32, kind="ExternalInput")
    b8 = nc.dram_tensor("b8", [128, NSLOT], F32, kind="ExternalInput")
    ls8 = nc.dram_tensor("ls8", [128, NSLOT], F32, kind="ExternalInput")
    on1 = nc.dram_tensor("on1", [128, 1], F32R, kind="ExternalInput")
    yt = nc.dram_tensor("yt", [1024, 1024], F32, kind="ExternalOutput")
    lj = nc.dram_tensor("lj", [128, NSLOT, BLK], F32, kind="ExternalOutput")
    srt = nc.dram_tensor("srt", [1024], F32)  # s roundtrip scratch

    with tile.TileContext(nc) as tc:
        with (
            tc.tile_pool(name="wp", bufs=1) as wp,
            tc.tile_pool(name="xp", bufs=8) as xp,
            tc.tile_pool(name="xmp", bufs=2) as xmp,
            tc.tile_pool(name="yp", bufs=4) as yp,
            tc.tile_pool(name="smallp", bufs=1) as smallp,
            tc.tile_pool(name="psy", bufs=6, space="PSUM") as pyp,
        ):
            xc_sb = {}

            def emit_xc(nt, c, eng=None):
                # split into 4 sub-DMAs so the transfer spreads across DGE
                # queues (more SDMA engines in parallel)
                xc = xp.tile([128, 8, NTW], F16, tag="xc", name=f"xc{nt}_{c}")
                e = eng or nc.sync
                for q in range(4):
                    e.dma_start(
                        out=xc[:, 2 * q : 2 * q + 2, :],
                        in_=xt[:, nt, c, 2 * q : 2 * q + 2, :],
                    )
                xc_sb[(nt, c)] = xc

            # ---- small constants (gpsimd queue; keeps sync free for bulk) ----
            ones = smallp.tile([128, 1], F32R)
            nc.gpsimd.dma_start(out=ones, in_=on1[:, :])
            dgt_sb = smallp.tile([128, NSLOT, BLK], F32)
            nc.gpsimd.dma_start(out=dgt_sb, in_=dgt[:, :, :])
            dgr_sb = smallp.tile([128, NSLOT, BLK], F32)
            nc.gpsimd.dma_start(out=dgr_sb, in_=dgr[:, :, :])
            b8_sb = smallp.tile([128, NSLOT], F32)
            nc.gpsimd.dma_start(out=b8_sb, in_=b8[:, :])
            ls8_sb = smallp.tile([128, NSLOT], F32)
            nc.gpsimd.dma_start(out=ls8_sb, in_=ls8[:, :])

            # compact exp of diagonal blocks: a_cmp[p, s, a] = exp(dgt[p, s, a])
            # in one ACT op, then scatter the 32x32 sub-blocks onto the
            # block-diagonal of a_exp with 4 SBUF->SBUF DMAs (zero elsewhere).
            a_cmp = smallp.tile([128, NSLOT, BLK], F16)
            nc.scalar.activation(
                a_cmp.rearrange("p s a -> p (s a)"),
                dgt_sb.rearrange("p s a -> p (s a)"),
                AF.Exp,
            )
            zmem = smallp.tile([128, NSLOT * 128], F32)
            nc.vector.memset(zmem, 0.0)
            a_exp = smallp.tile([128, NSLOT, 128], F16)
            nc.vector.tensor_copy(a_exp.rearrange("p s f -> p (s f)"), zmem)
            for j in range(4):
                pr = slice(32 * j, 32 * j + 32)
                nc.gpsimd.dma_start(out=a_exp[pr, :, 32 * j : 32 * j + 32], in_=a_cmp[pr, :, :])

            w_chunk = []

            def w_tile(kt, s):
                t = T_IDX[(kt, s)]
                c = kt // 4
                t0, _ = _wc_range(c)
                return w_chunk[c][:, (t - t0) * 128 : (t - t0 + 1) * 128]

            def emit_matmuls(s, nt, out_sb, copy_raw):
                """Accumulate slot s, n-tile nt into PSUM, then copy to out_sb.

                copy_raw=True: plain PSUM->SBUF copy (g not available yet);
                the g*psum+bias affine is applied later in place.
                copy_raw=False: fused g*psum+bias copyback.
                """
                xms = xmp.tile([128, NTW], F16, tag="xm", name=f"xm{nt}_{s}")
                nc.gpsimd.dma_start(out=xms, in_=xm[:, nt, s, :])
                ps_y = pyp.tile([128, NTW], F32, tag="psy")
                for kt in range(4 * (s + 1)):
                    nc.tensor.matmul(
                        ps_y,
                        w_tile(kt, s),
                        xc_sb[(nt, kt // 8)][:, kt % 8, :],
                        start=(kt == 0),
                        stop=False,
                    )
                # diag-block exp contribution (block-diagonal lhsT, full K)
                nc.tensor.matmul(
                    ps_y, a_exp[:, s, :], xms, start=False, stop=True
                )
                if copy_raw:
                    nc.scalar.copy(out_sb, ps_y)
                else:
                    nc.scalar.activation(
                        out_sb, ps_y, AF.Identity,
                        bias=b8_sb[:, s : s + 1], scale=g_sb[:, s : s + 1],
                    )

            g_sb = smallp.tile([128, NSLOT], F32)
            ljb = smallp.tile([128, NSLOT], F32)
            # staging for nt=0 results (copied raw before g is known)
            stage = [
                smallp.tile([128, NTW], F32, tag=f"st{s}", name=f"stage{s}")
                for s in range(NSLOT)
            ]

            # ---- interleaved: W chunk DMA -> norm jobs -> main slot (nt=0) ----
            # DMA triggers are emitted in consumption order (wc0, xc0, wc1,
            # wc2, xc1, ...) so early slots' data arrives first; main slot
            # emission lags the chunk loop by 2 so the PE never parks on the
            # a_exp-dependent tail matmul while W/x are still streaming.
            with (
                tc.tile_pool(name="psn", bufs=1, space="PSUM") as pnp,
                tc.tile_pool(name="sqp", bufs=2) as sqp,
            ):
                psn = pnp.tile([1, 1024], F32)

                # W chunk tiles created up front; chunks 0-3 DMA'd immediately
                # (first wave), chunks 4-7 triggered from the ACT engine after
                # early slots' copybacks so DMA issue paces with compute.
                for c in range(NWC):
                    t0c, t1c = _wc_range(c)
                    wc = wp.tile([128, (t1c - t0c) * 128], F16, tag=f"wc{c}", name=f"wct{c}")
                    w_chunk.append(wc)

                def trigger_wc(c, eng):
                    t0c, t1c = _wc_range(c)
                    # split in 2 sub-DMAs for queue spread
                    mid = (t0c + t1c) // 2
                    eng.dma_start(
                        out=w_chunk[c][:, : (mid - t0c) * 128],
                        in_=ws[:, t0c * 128 : mid * 128],
                    )
                    eng.dma_start(
                        out=w_chunk[c][:, (mid - t0c) * 128 :],
                        in_=ws[:, mid * 128 : t1c * 128],
                    )

                # everything up front, consumption-ordered; the DGE round-robin
                # shares bandwidth, order mostly sets start order
                trigger_wc(0, nc.sync)
                emit_xc(0, 0)
                trigger_wc(1, nc.sync)
                emit_xc(0, 1)
                trigger_wc(2, nc.sync)
                trigger_wc(3, nc.sync)
                emit_xc(0, 2)
                trigger_wc(4, nc.sync)
                trigger_wc(5, nc.sync)
                emit_xc(0, 3)
                trigger_wc(6, nc.sync)
                trigger_wc(7, nc.sync)

                for c in range(NWC):
                    c0 = _wc_range(c)[0]
                    for kt in range(4 * c, 4 * c + 4):
                        t0, t1 = _kt_range(kt)
                        nact = t1 - t0
                        w_slice = w_chunk[c][:, (t0 - c0) * 128 : (t1 - c0) * 128]
                        smin = c * 128
                        off = 0
                        while off < nact * 128:
                            n = min(512, nact * 128 - off)
                            sq = sqp.tile([128, 512], F32R, tag="sq")
                            nc.vector.tensor_mul(
                                sq[:, :n],
                                w_slice[:, off : off + n],
                                w_slice[:, off : off + n],
                            )
                            nc.tensor.matmul(
                                psn[0:1, smin + off : smin + off + n],
                                ones,
                                sq[:, :n],
                                start=(kt == 0),
                                stop=False,
                            )
                            off += n
                    # lag main-slot emission 2 chunks behind the DMA stream
                    if c >= 2:
                        emit_matmuls(c - 2, 0, stage[c - 2], copy_raw=True)
                for s in range(NSLOT - 2, NSLOT):
                    emit_matmuls(s, 0, stage[s], copy_raw=True)
                # diag-block contribution to the norms (block-diagonal squares)
                ae_flat = a_exp.rearrange("p s f -> p (s f)")
                for half in range(2):
                    a_sq = sqp.tile([128, 512], F32R, tag="sq", name=f"asq{half}")
                    nc.vector.tensor_mul(
                        a_sq, ae_flat[:, half * 512 : (half + 1) * 512],
                        ae_flat[:, half * 512 : (half + 1) * 512],
                    )
                    nc.tensor.matmul(
                        psn[0:1, half * 512 : (half + 1) * 512], ones, a_sq,
                        start=False, stop=(half == 1),
                    )
                s_row = smallp.tile([1, 1024], F32)
                nc.scalar.copy(s_row, psn[0:1, :])

            # s roundtrip through DRAM to get per-partition layout [128, 8]
            nc.gpsimd.dma_start(out=srt[:].rearrange("(a n) -> a n", a=1), in_=s_row[0:1, :])
            s_col = smallp.tile([128, NSLOT], F32)
            nc.gpsimd.dma_start(out=s_col, in_=srt[:].rearrange("(s p) -> p s", p=128))

            # g = exp(ls - 0.5 ln s);  ljb = ls - 0.5 ln s
            t_ln = smallp.tile([128, NSLOT], F32)
            nc.scalar.activation(t_ln, s_col, AF.Ln)
            for s in range(NSLOT):
                nc.scalar.activation(
                    ljb[:, s : s + 1], t_ln[:, s : s + 1], AF.Identity,
                    bias=ls8_sb[:, s : s + 1], scale=-0.5,
                )
            nc.scalar.activation(g_sb, ljb, AF.Exp)

            # log-jacobian: lj = W_diag + (ls - 0.5 ln s)
            lj_sb = smallp.tile([128, NSLOT, BLK], F32)
            for s in range(NSLOT):
                nc.scalar.activation(
                    lj_sb[:, s, :], dgr_sb[:, s, :], AF.Identity,
                    bias=ljb[:, s : s + 1], scale=1.0,
                )
            nc.gpsimd.dma_start(out=lj[:, :, :], in_=lj_sb)

            # nt=0: apply y = g*acc + bias in place, then store
            for s in range(NSLOT):
                nc.scalar.activation(
                    stage[s], stage[s], AF.Identity,
                    bias=b8_sb[:, s : s + 1], scale=g_sb[:, s : s + 1],
                )
                nc.scalar.dma_start(
                    out=yt[s * 128 : (s + 1) * 128, 0:NTW], in_=stage[s]
                )

            # ---- second batch half (nt=1) ----
            for c in range(NXC):
                emit_xc(1, c)
            for s in range(NSLOT):
                y_sb = yp.tile([128, NTW], F32, tag="y")
                emit_matmuls(s, 1, y_sb, copy_raw=False)
                nc.scalar.dma_start(
                    out=yt[s * 128 : (s + 1) * 128, NTW : 2 * NTW], in_=y_sb
                )
    nc.finalize()
    return nc


def _pack_inputs(x, W, bias, W_log_scale):
    """Host-side data layout: slice/transpose/zero-select only (no math)."""
    f32 = np.float32
    WT = np.ascontiguousarray(W.T.astype(f32, copy=False))
    xT = np.ascontiguousarray(x.T.astype(f32, copy=False))
    bias = bias.astype(f32, copy=False)
    ls = W_log_scale.astype(f32, copy=False)

    grp = {}
    for g in range(NGRP):
        ws_g = np.zeros((128, NSTREAM, 128), np.float16)
        for t, (kt, s) in enumerate(STREAM):
            mg = 4 * s + g
            if kt < mg:
                ws_g[:, t, :] = WT[kt * 128 : (kt + 1) * 128, mg * 128 : (mg + 1) * 128]
            elif kt == mg:
                blk = WT[kt * 128 : (kt + 1) * 128, mg * 128 : (mg + 1) * 128].copy()
                for jp in range(4):  # keep only sub-blocks strictly above the diagonal
                    blk[32 * jp : 32 * jp + 32, : 32 * (jp + 1)] = 0.0
                ws_g[:, t, :] = blk
            # kt > mg: zero pad
        dgt = np.empty((128, NSLOT, BLK), f32)
        dgr = np.empty((128, NSLOT, BLK), f32)
        for s in range(NSLOT):
            mg = 4 * s + g
            blk = W[mg * 128 : (mg + 1) * 128, mg * 128 : (mg + 1) * 128]
            for j in range(4):
                sub = blk[32 * j : 32 * j + 32, 32 * j : 32 * j + 32]  # [a, b]
                dgt[32 * j : 32 * j + 32, s, :] = sub.T
                dgr[32 * j : 32 * j + 32, s, :] = sub
        b8 = np.stack(
            [bias[(4 * s + g) * 128 : (4 * s + g + 1) * 128] for s in range(NSLOT)], 1
        )
        ls8 = np.stack(
            [ls[(4 * s + g) * 128 : (4 * s + g + 1) * 128, 0] for s in range(NSLOT)], 1
        )
        grp[g] = dict(
            ws=np.ascontiguousarray(ws_g.reshape(128, NSTREAM * 128)),
            dgt=dgt, dgr=dgr, b8=np.ascontiguousarray(b8), ls8=np.ascontiguousarray(ls8),
        )

    # x in per-core SBUF layout: xt[p, nt, c, j, n] = xT[(8c+j)*128+p, h*1024+nt*512+n]
    xt_h = {}
    xm_h = {}
    for h in range(2):
        xs = xT[:, h * 1024 : (h + 1) * 1024]               # [4096, 1024]
        v = xs.reshape(KT, 128, NT, NTW)                     # [kt, p, nt, n]
        v = v.transpose(1, 2, 0, 3)                          # [p, nt, kt, n]
        xt_h[h] = np.ascontiguousarray(v.reshape(128, NT, NXC, 8, NTW).astype(np.float16))
    in_maps = []
    for c in range(8):
        g, h = c // 2, c % 2
        xm_c = np.empty((128, NT, NSLOT, NTW), np.float16)
        for s in range(NSLOT):
            mg = 4 * s + g
            for nt in range(NT):
                xm_c[:, nt, s, :] = xT[
                    mg * 128 : (mg + 1) * 128,
                    h * 1024 + nt * NTW : h * 1024 + (nt + 1) * NTW,
                ]
        in_maps.append(
            {
                **grp[g],
                "xt": xt_h[h],
                "xm": xm_c,
                "on1": np.ones((128, 1), f32),
            }
        )
    return in_maps


_PROGRAM = None


def kernel(x, W, bias, W_log_scale, b_diag_mask=None, b_tril_mask=None, **_ignored):
    """Full inputs in, full outputs out. Returns (y, log_jac) like the reference.

    The diag/tril masks are the fixed block-kronecker patterns of the module
    (block-diagonal + strict block-lower-triangular); the kernel exploits that
    structure directly, so the mask tensors themselves are not transferred.
    """
    from concourse.bass_utils import run_bass_kernel_spmd

    global _PROGRAM
    if _PROGRAM is None:
        _PROGRAM = _build_program()
    nc = _PROGRAM

    in_maps = _pack_inputs(
        np.asarray(x), np.asarray(W), np.asarray(bias), np.asarray(W_log_scale)
    )
    res = run_bass_kernel_spmd(nc, in_maps, list(range(8)))

    y = np.empty((BATCH, DIM), np.float32)
    lj_full = np.empty((NB, BLK, BLK), np.float32)
    for c, r in enumerate(res.results):
        g, h = c // 2, c % 2
        ytc = r["yt"]
        for s in range(NSLOT):
            mg = 4 * s + g
            y[h * 1024 : (h + 1) * 1024, mg * 128 : (mg + 1) * 128] = (
                ytc[s * 128 : (s + 1) * 128, :].T
            )
        if h == 0:
            ljc = r["lj"]  # [128, 8, 32]
            for s in range(NSLOT):
                mg = 4 * s + g
                for j in range(4):
                    lj_full[4 * mg + j] = ljc[32 * j : 32 * j + 32, s, :]
    return (y, lj_full)


# revision 52
# speedup vs baseline: 1.6485x; 1.0000x over previous
"""Trainium2 Bass kernel for BlockAutoregressiveLinear (n_blocks=128, B=32,
in=out=4096, batch=2048), 8 NeuronCores.

Math (see reference):
    Wm = exp(W)*diag_mask + W*tril_mask          (block-diag exp, strict block-lower copy)
    s[o] = sum_i Wm[o,i]^2
    g[o] = exp(W_log_scale[o] - 0.5*ln s[o])
    y = x @ (g[:,None]*Wm).T + bias
    log_jac[k,a,b] = W[k*32+a, k*32+b] + W_log_scale[o] - 0.5*ln s[o]

Sharding: 4 column groups x 2 batch halves. Core c=(2g+h) owns 8 output
m-tiles (128 rows each) mg = 4s+g for slot s=0..7 and batch half h.
W is block-lower-triangular, so m-tile mg only needs k-tiles 0..mg. For an
SPMD-uniform instruction stream, slot s is padded to K_s = 4(s+1) k-tile
matmuls (pad tiles are host-packed zeros); every core executes the identical
program and all per-core variation lives in the input data.

The diagonal-block exp contribution is a separate per-slot matmul (a_exp
tile built on device with ACT Exp) against a host-packed copy of the
corresponding x rows (xm), so the stream tiles are pure copies of W.T
(masked on host by zeroing = data selection; all arithmetic is on device).

Main matmuls run in fp16 (the weight-normalization cancels most of the
W-quantization error; measured end-to-end rel err ~3e-4, same class as
fp32r) with N=512 moving tiles; the row-norm reduction runs in f32r.
The weight scaling by g and the bias add are fused into the PSUM->SBUF
copyback on the ACT engine (per-partition scale/bias).
"""

import sys

sys.path.insert(0, "/opt/trn_rl_repo")

import numpy as np

NB = 128          # autoregressive blocks
BLK = 32          # block size
DIM = 4096
BATCH = 2048
NSLOT = 8         # m-tile slots per core
NGRP = 4          # column groups
NT = 2            # n-tiles per batch half
NTW = 512         # n-tile width
KT = 32           # k tiles
NWC = 8           # W stream chunks (4 k-tiles each)
NXC = 4           # x chunks per n-tile (8 k-tiles each)

# k-major stream of (kt, slot) jobs; slot s is active for kt < 4(s+1)
STREAM = [(kt, s) for kt in range(KT) for s in range(NSLOT) if kt < 4 * (s + 1)]
T_IDX = {j: t for t, j in enumerate(STREAM)}
NSTREAM = len(STREAM)  # 144


def _kt_range(kt):
    """[t0, t1) tile-index range of stream tiles belonging to k-tile kt."""
    t0 = T_IDX[(kt, kt // 4)]
    return t0, t0 + NSLOT - kt // 4


def _wc_range(c):
    """[t0, t1) tile-index range of W chunk c (k-tiles 4c..4c+3)."""
    return _kt_range(4 * c)[0], _kt_range(4 * c + 3)[1]


def _build_program():
    import concourse.tile as tile
    from concourse import bacc, mybir

    F32 = mybir.dt.float32
    F32R = mybir.dt.float32r
    F16 = mybir.dt.float16
    AF = mybir.ActivationFunctionType

    nc = bacc.Bacc("TRN2", target_bir_lowering=False, debug=False, num_devices=8)

    ws = nc.dram_tensor("ws", [128, NSTREAM * 128], F16, kind="ExternalInput")
    # x, host-packed per-core in SBUF layout: [p, nt, xchunk, ktile-in-chunk, n]
    xt = nc.dram_tensor("xt", [128, NT, NXC, 8, NTW], F16, kind="ExternalInput")
    xm = nc.dram_tensor("xm", [128, NT, NSLOT, NTW], F16, kind="ExternalInput")
    dgt = nc.dram_tensor("dgt", [128, NSLOT, BLK], F32, kind="ExternalInput")
    dgr = nc.dram_tensor("dgr", [128, NSLOT, BLK], F32, kind="ExternalInput")
    b8 = nc.dram_tensor("b8", [128, NSLOT], F32, kind="ExternalInput")
    ls8 = nc.dram_tensor("ls8", [128, NSLOT], F32, kind="ExternalInput")
    on1 = nc.dram_tensor("on1", [128, 1], F32R, kind="ExternalInput")
    yt = nc.dram_tensor("yt", [1024, 1024], F32, kind="ExternalOutput")
    lj = nc.dram_tensor("lj", [128, NSLOT, BLK], F32, kind="ExternalOutput")
    srt = nc.dram_tensor("srt", [1024], F32)  # s roundtrip scratch

    with tile.TileContext(nc) as tc:
        with (
            tc.tile_pool(name="wp", bufs=1) as wp,
            tc.tile_pool(name="xp", bufs=8) as xp,
            tc.tile_pool(name="xmp", bufs=4) as xmp,
            tc.tile_pool(name="yp", bufs=4) as yp,
            tc.tile_pool(name="smallp", bufs=1) as smallp,
            tc.tile_pool(name="psy", bufs=6, space="PSUM") as pyp,
        ):
            xc_sb = {}

            def emit_xc(nt, c, eng=None):
                # split into 4 sub-DMAs so the transfer spreads across DGE
                # queues (more SDMA engines in parallel)
                xc = xp.tile([128, 8, NTW], F16, tag="xc", name=f"xc{nt}_{c}")
                e = eng or nc.sync
                for q in range(4):
                    e.dma_start(
                        out=xc[:, 2 * q : 2 * q + 2, :],
                        in_=xt[:, nt, c, 2 * q : 2 * q + 2, :],
                    )
                xc_sb[(nt, c)] = xc

            # ---- small constants (gpsimd queue; keeps sync free for bulk) ----
            ones = smallp.tile([128, 1], F32R)
            nc.gpsimd.dma_start(out=ones, in_=on1[:, :])
            dgt_sb = smallp.tile([128, NSLOT, BLK], F32)
            nc.gpsimd.dma_start(out=dgt_sb, in_=dgt[:, :, :])
            dgr_sb = smallp.tile([128, NSLOT, BLK], F32)
            nc.gpsimd.dma_start(out=dgr_sb, in_=dgr[:, :, :])
            b8_sb = smallp.tile([128, NSLOT], F32)
            nc.gpsimd.dma_start(out=b8_sb, in_=b8[:, :])
            ls8_sb = smallp.tile([128, NSLOT], F32)
            nc.gpsimd.dma_start(out=ls8_sb, in_=ls8[:, :])

            # compact exp of diagonal blocks: a_cmp[p, s, a] = exp(dgt[p, s, a])
            # in one ACT op, then scatter the 32x32 sub-blocks onto the
            # block-diagonal of a_exp with 4 SBUF->SBUF DMAs (zero elsewhere).
            a_cmp = smallp.tile([128, NSLOT, BLK], F16)
            nc.scalar.activation(
                a_cmp.rearrange("p s a -> p (s a)"),
                dgt_sb.rearrange("p s a -> p (s a)"),
                AF.Exp,
            )
            zmem = smallp.tile([128, NSLOT * 128], F32)
            nc.vector.memset(zmem, 0.0)
            a_exp = smallp.tile([128, NSLOT, 128], F16)
            nc.vector.tensor_copy(a_exp.rearrange("p s f -> p (s f)"), zmem)
            for j in range(4):
                pr = slice(32 * j, 32 * j + 32)
                nc.gpsimd.dma_start(out=a_exp[pr, :, 32 * j : 32 * j + 32], in_=a_cmp[pr, :, :])

            w_chunk = []

            def w_tile(kt, s):
                t = T_IDX[(kt, s)]
                c = kt // 4
                t0, _ = _wc_range(c)
                return w_chunk[c][:, (t - t0) * 128 : (t - t0 + 1) * 128]

            def emit_matmuls(s, nt, out_sb, copy_raw):
                """Accumulate slot s, n-tile nt into PSUM, then copy to out_sb.

                copy_raw=True: plain PSUM->SBUF copy (g not available yet);
                the g*psum+bias affine is applied later in place.
                copy_raw=False: fused g*psum+bias copyback.
                """
                xms = xmp.tile([128, NTW], F16, tag="xm", name=f"xm{nt}_{s}")
                nc.gpsimd.dma_start(out=xms, in_=xm[:, nt, s, :])
                ps_y = pyp.tile([128, NTW], F32, tag="psy")
                for kt in range(4 * (s + 1)):
                    nc.tensor.matmul(
                        ps_y,
                        w_tile(kt, s),
                        xc_sb[(nt, kt // 8)][:, kt % 8, :],
                        start=(kt == 0),
                        stop=False,
                    )
                # diag-block exp contribution (block-diagonal lhsT, full K)
                nc.tensor.matmul(
                    ps_y, a_exp[:, s, :], xms, start=False, stop=True
                )
                if copy_raw:
                    nc.scalar.copy(out_sb, ps_y)
                else:
                    nc.scalar.activation(
                        out_sb, ps_y, AF.Identity,
                        bias=b8_sb[:, s : s + 1], scale=g_sb[:, s : s + 1],
                    )

            g_sb = smallp.tile([128, NSLOT], F32)
            ljb = smallp.tile([128, NSLOT], F32)
            # staging for nt=0 results (copied raw before g is known)
            stage = [
                smallp.tile([128, NTW], F32, tag=f"st{s}", name=f"stage{s}")
                for s in range(NSLOT)
            ]

            # ---- interleaved: W chunk DMA -> norm jobs -> main slot (nt=0) ----
            # DMA triggers are emitted in consumption order (wc0, xc0, wc1,
            # wc2, xc1, ...) so early slots' data arrives first; main slot
            # emission lags the chunk loop by 2 so the PE never parks on the
            # a_exp-dependent tail matmul while W/x are still streaming.
            with (
                tc.tile_pool(name="psn", bufs=1, space="PSUM") as pnp,
                tc.tile_pool(name="sqp", bufs=2) as sqp,
            ):
                psn = pnp.tile([1, 1024], F32)

                # W chunk tiles created up front; chunks 0-3 DMA'd immediately
                # (first wave), chunks 4-7 triggered from the ACT engine after
                # early slots' copybacks so DMA issue paces with compute.
                for c in range(NWC):
                    t0c, t1c = _wc_range(c)
                    wc = wp.tile([128, (t1c - t0c) * 128], F16, tag=f"wc{c}", name=f"wct{c}")
                    w_chunk.append(wc)

                def trigger_wc(c, eng):
                    t0c, t1c = _wc_range(c)
                    # split in 2 sub-DMAs for queue spread
                    mid = (t0c + t1c) // 2
                    eng.dma_start(
                        out=w_chunk[c][:, : (mid - t0c) * 128],
                        in_=ws[:, t0c * 128 : mid * 128],
                    )
                    eng.dma_start(
                        out=w_chunk[c][:, (mid - t0c) * 128 :],
                        in_=ws[:, mid * 128 : t1c * 128],
                    )

                # everything up front, consumption-ordered; the DGE round-robin
                # shares bandwidth, order mostly sets start order
                trigger_wc(0, nc.sync)
                emit_xc(0, 0)
                trigger_wc(1, nc.sync)
                emit_xc(0, 1)
                trigger_wc(2, nc.sync)
                trigger_wc(3, nc.sync)
                emit_xc(0, 2)
                trigger_wc(4, nc.sync)
                trigger_wc(5, nc.sync)
                emit_xc(0, 3)
                trigger_wc(6, nc.sync)
                trigger_wc(7, nc.sync)
                for c in range(NXC):  # nt=1 x; all 8 chunks fit in SBUF at fp16
                    emit_xc(1, c)

                for c in range(NWC):
                    c0 = _wc_range(c)[0]
                    for kt in range(4 * c, 4 * c + 4):
                        t0, t1 = _kt_range(kt)
                        nact = t1 - t0
                        w_slice = w_chunk[c][:, (t0 - c0) * 128 : (t1 - c0) * 128]
                        smin = c * 128
                        off = 0
                        while off < nact * 128:
                            n = min(512, nact * 128 - off)
                            sq = sqp.tile([128, 512], F32R, tag="sq")
                            nc.vector.tensor_mul(
                                sq[:, :n],
                                w_slice[:, off : off + n],
                                w_slice[:, off : off + n],
                            )
                            nc.tensor.matmul(
                                psn[0:1, smin + off : smin + off + n],
                                ones,
                                sq[:, :n],
                                start=(kt == 0),
                                stop=False,
                            )
                            off += n
                    # lag main-slot emission 2 chunks behind the DMA stream
                    if c >= 2:
                        emit_matmuls(c - 2, 0, stage[c - 2], copy_raw=True)
                for s in range(NSLOT - 2, NSLOT):
                    emit_matmuls(s, 0, stage[s], copy_raw=True)
                # diag-block contribution to the norms (block-diagonal squares)
                ae_flat = a_exp.rearrange("p s f -> p (s f)")
                for half in range(2):
                    a_sq = sqp.tile([128, 512], F32R, tag="sq", name=f"asq{half}")
                    nc.vector.tensor_mul(
                        a_sq, ae_flat[:, half * 512 : (half + 1) * 512],
                        ae_flat[:, half * 512 : (half + 1) * 512],
                    )
                    nc.tensor.matmul(
                        psn[0:1, half * 512 : (half + 1) * 512], ones, a_sq,
                        start=False, stop=(half == 1),
                    )
                s_row = smallp.tile([1, 1024], F32)
                nc.scalar.copy(s_row, psn[0:1, :])

            # s roundtrip through DRAM to get per-partition layout [128, 8]
            nc.gpsimd.dma_start(out=srt[:].rearrange("(a n) -> a n", a=1), in_=s_row[0:1, :])
            s_col = smallp.tile([128, NSLOT], F32)
            nc.gpsimd.dma_start(out=s_col, in_=srt[:].rearrange("(s p) -> p s", p=128))

            # g = exp(ls - 0.5 ln s);  ljb = ls - 0.5 ln s
            t_ln = smallp.tile([128, NSLOT], F32)
            nc.scalar.activation(t_ln, s_col, AF.Ln)
            for s in range(NSLOT):
                nc.scalar.activation(
                    ljb[:, s : s + 1], t_ln[:, s : s + 1], AF.Identity,
                    bias=ls8_sb[:, s : s + 1], scale=-0.5,
                )
            nc.scalar.activation(g_sb, ljb, AF.Exp)

            # log-jacobian: lj = W_diag + (ls - 0.5 ln s)
            lj_sb = smallp.tile([128, NSLOT, BLK], F32)
            for s in range(NSLOT):
                nc.scalar.activation(
                    lj_sb[:, s, :], dgr_sb[:, s, :], AF.Identity,
                    bias=ljb[:, s : s + 1], scale=1.0,
                )
            nc.gpsimd.dma_start(out=lj[:, :, :], in_=lj_sb)

            # nt=0: apply y = g*acc + bias in place, then store
            for s in range(NSLOT):
                nc.scalar.activation(
                    stage[s], stage[s], AF.Identity,
                    bias=b8_sb[:, s : s + 1], scale=g_sb[:, s : s + 1],
                )
                nc.scalar.dma_start(
                    out=yt[s * 128 : (s + 1) * 128, 0:NTW], in_=stage[s]
                )

            # ---- second batch half (nt=1; x already resident) ----
            for s in range(NSLOT):
                y_sb = yp.tile([128, NTW], F32, tag="y")
                emit_matmuls(s, 1, y_sb, copy_raw=False)
                nc.scalar.dma_start(
                    out=yt[s * 128 : (s + 1) * 128, NTW : 2 * NTW], in_=y_sb
                )
    nc.finalize()
    return nc


def _pack_inputs(x, W, bias, W_log_scale):
    """Host-side data layout: slice/transpose/zero-select only (no math)."""
    f32 = np.float32
    WT = np.ascontiguousarray(W.T.astype(f32, copy=False))
    xT = np.ascontiguousarray(x.T.astype(f32, copy=False))
    bias = bias.astype(f32, copy=False)
    ls = W_log_scale.astype(f32, copy=False)

    grp = {}
    for g in range(NGRP):
        ws_g = np.zeros((128, NSTREAM, 128), np.float16)
        for t, (kt, s) in enumerate(STREAM):
            mg = 4 * s + g
            if kt < mg:
                ws_g[:, t, :] = WT[kt * 128 : (kt + 1) * 128, mg * 128 : (mg + 1) * 128]
            elif kt == mg:
                blk = WT[kt * 128 : (kt + 1) * 128, mg * 128 : (mg + 1) * 128].copy()
                for jp in range(4):  # keep only sub-blocks strictly above the diagonal
                    blk[32 * jp : 32 * jp + 32, : 32 * (jp + 1)] = 0.0
                ws_g[:, t, :] = blk
            # kt > mg: zero pad
        dgt = np.empty((128, NSLOT, BLK), f32)
        dgr = np.empty((128, NSLOT, BLK), f32)
        for s in range(NSLOT):
            mg = 4 * s + g
            blk = W[mg * 128 : (mg + 1) * 128, mg * 128 : (mg + 1) * 128]
            for j in range(4):
                sub = blk[32 * j : 32 * j + 32, 32 * j : 32 * j + 32]  # [a, b]
                dgt[32 * j : 32 * j + 32, s, :] = sub.T
                dgr[32 * j : 32 * j + 32, s, :] = sub
        b8 = np.stack(
            [bias[(4 * s + g) * 128 : (4 * s + g + 1) * 128] for s in range(NSLOT)], 1
        )
        ls8 = np.stack(
            [ls[(4 * s + g) * 128 : (4 * s + g + 1) * 128, 0] for s in range(NSLOT)], 1
        )
        grp[g] = dict(
            ws=np.ascontiguousarray(ws_g.reshape(128, NSTREAM * 128)),
            dgt=dgt, dgr=dgr, b8=np.ascontiguousarray(b8), ls8=np.ascontiguousarray(ls8),
        )

    # x in per-core SBUF layout: xt[p, nt, c, j, n] = xT[(8c+j)*128+p, h*1024+nt*512+n]
    xt_h = {}
    xm_h = {}
    for h in range(2):
        xs = xT[:, h * 1024 : (h + 1) * 1024]               # [4096, 1024]
        v = xs.reshape(KT, 128, NT, NTW)                     # [kt, p, nt, n]
        v = v.transpose(1, 2, 0, 3)                          # [p, nt, kt, n]
        xt_h[h] = np.ascontiguousarray(v.reshape(128, NT, NXC, 8, NTW).astype(np.float16))
    in_maps = []
    for c in range(8):
        g, h = c // 2, c % 2
        xm_c = np.empty((128, NT, NSLOT, NTW), np.float16)
        for s in range(NSLOT):
            mg = 4 * s + g
            for nt in range(NT):
                xm_c[:, nt, s, :] = xT[
                    mg * 128 : (mg + 1) * 128,
                    h * 1024 + nt * NTW : h * 1024 + (nt + 1) * NTW,
                ]
        in_maps.append(
            {
                **grp[g],
                "xt": xt_h[h],
                "xm": xm_c,
                "on1": np.ones((128, 1), f32),
            }
        )
    return in_maps


_PROGRAM = None


def kernel(x, W, bias, W_log_scale, b_diag_mask=None, b_tril_mask=None, **_ignored):
    """Full inputs in, full outputs out. Returns (y, log_jac) like the reference.

    The diag/tril masks are the fixed block-kronecker patterns of the module
    (block-diagonal + strict block-lower-triangular); the kernel exploits that
    structure directly, so the mask tensors themselves are not transferred.
    """
    from concourse.bass_utils import run_bass_kernel_spmd

    global _PROGRAM
    if _PROGRAM is None:
        _PROGRAM = _build_program()
    nc = _PROGRAM

    in_maps = _pack_inputs(
        np.asarray(x), np.asarray(W), np.asarray(bias), np.asarray(W_log_scale)
    )
    res = run_bass_kernel_spmd(nc, in_maps, list(range(8)))

    y = np.empty((BATCH, DIM), np.float32)
    lj_full = np.empty((NB, BLK, BLK), np.float32)
    for c, r in enumerate(res.results):
        g, h = c // 2, c % 2
        ytc = r["yt"]
        for s in range(NSLOT):
            mg = 4 * s + g
            y[h * 1024 : (h + 1) * 1024, mg * 128 : (mg + 1) * 128] = (
                ytc[s * 128 : (s + 1) * 128, :].T
            )
        if h == 0:
            ljc = r["lj"]  # [128, 8, 32]
            for s in range(NSLOT):
                mg = 4 * s + g
                for j in range(4):
                    lj_full[4 * mg + j] = ljc[32 * j : 32 * j + 32, s, :]
    return (y, lj_full)


# revision 54
# speedup vs baseline: 1.7501x; 1.0617x over previous
"""Trainium2 Bass kernel for BlockAutoregressiveLinear (n_blocks=128, B=32,
in=out=4096, batch=2048), 8 NeuronCores.

Math (see reference):
    Wm = exp(W)*diag_mask + W*tril_mask          (block-diag exp, strict block-lower copy)
    s[o] = sum_i Wm[o,i]^2
    g[o] = exp(W_log_scale[o] - 0.5*ln s[o])
    y = x @ (g[:,None]*Wm).T + bias
    log_jac[k,a,b] = W[k*32+a, k*32+b] + W_log_scale[o] - 0.5*ln s[o]

Sharding: 4 column groups x 2 batch halves. Core c=(2g+h) owns 8 output
m-tiles (128 rows each) mg = 4s+g for slot s=0..7 and batch half h.
W is block-lower-triangular, so m-tile mg only needs k-tiles 0..mg. For an
SPMD-uniform instruction stream, slot s is padded to K_s = 4(s+1) k-tile
matmuls (pad tiles are host-packed zeros); every core executes the identical
program and all per-core variation lives in the input data.

The diagonal-block exp contribution is a separate per-slot matmul (a_exp
tile built on device with ACT Exp) against a host-packed copy of the
corresponding x rows (xm), so the stream tiles are pure copies of W.T
(masked on host by zeroing = data selection; all arithmetic is on device).

Main matmuls run in fp16 (the weight-normalization cancels most of the
W-quantization error; measured end-to-end rel err ~3e-4, same class as
fp32r) with N=512 moving tiles; the row-norm reduction runs in f32r.
The weight scaling by g and the bias add are fused into the PSUM->SBUF
copyback on the ACT engine (per-partition scale/bias).
"""

import sys

sys.path.insert(0, "/opt/trn_rl_repo")

import numpy as np

NB = 128          # autoregressive blocks
BLK = 32          # block size
DIM = 4096
BATCH = 2048
NSLOT = 8         # m-tile slots per core
NGRP = 4          # column groups
NT = 2            # n-tiles per batch half
NTW = 512         # n-tile width
KT = 32           # k tiles
NWC = 8           # W stream chunks (4 k-tiles each)
NXC = 4           # x chunks per n-tile (8 k-tiles each)

# k-major stream of (kt, slot) jobs; slot s is active for kt < 4(s+1)
STREAM = [(kt, s) for kt in range(KT) for s in range(NSLOT) if kt < 4 * (s + 1)]
T_IDX = {j: t for t, j in enumerate(STREAM)}
NSTREAM = len(STREAM)  # 144


def _kt_range(kt):
    """[t0, t1) tile-index range of stream tiles belonging to k-tile kt."""
    t0 = T_IDX[(kt, kt // 4)]
    return t0, t0 + NSLOT - kt // 4


def _wc_range(c):
    """[t0, t1) tile-index range of W chunk c (k-tiles 4c..4c+3)."""
    return _kt_range(4 * c)[0], _kt_range(4 * c + 3)[1]


def _build_program():
    import concourse.tile as tile
    from concourse import bacc, mybir

    F32 = mybir.dt.float32
    F32R = mybir.dt.float32r
    F16 = mybir.dt.float16
    AF = mybir.ActivationFunctionType

    nc = bacc.Bacc("TRN2", target_bir_lowering=False, debug=False, num_devices=8)

    ws = nc.dram_tensor("ws", [128, NSTREAM * 128], F16, kind="ExternalInput")
    # x, host-packed per-core in SBUF layout: [p, nt, xchunk, ktile-in-chunk, n]
    xt = nc.dram_tensor("xt", [128, NT, NXC, 8, NTW], F16, kind="ExternalInput")
    xm = nc.dram_tensor("xm", [128, NT, NSLOT, NTW], F16, kind="ExternalInput")
    dgt = nc.dram_tensor("dgt", [128, NSLOT, BLK], F32, kind="ExternalInput")
    dgr = nc.dram_tensor("dgr", [128, NSLOT, BLK], F32, kind="ExternalInput")
    b8 = nc.dram_tensor("b8", [128, NSLOT], F32, kind="ExternalInput")
    ls8 = nc.dram_tensor("ls8", [128, NSLOT], F32, kind="ExternalInput")
    on1 = nc.dram_tensor("on1", [128, 1], F32R, kind="ExternalInput")
    yt = nc.dram_tensor("yt", [1024, 1024], F32, kind="ExternalOutput")
    lj = nc.dram_tensor("lj", [128, NSLOT, BLK], F32, kind="ExternalOutput")
    srt = nc.dram_tensor("srt", [1024], F32)  # s roundtrip scratch

    with tile.TileContext(nc) as tc:
        with (
            tc.tile_pool(name="wp", bufs=1) as wp,
            tc.tile_pool(name="xp", bufs=8) as xp,
            tc.tile_pool(name="xmp", bufs=4) as xmp,
            tc.tile_pool(name="yp", bufs=4) as yp,
            tc.tile_pool(name="smallp", bufs=1) as smallp,
            tc.tile_pool(name="psy", bufs=6, space="PSUM") as pyp,
        ):
            xc_sb = {}

            def emit_xc(nt, c, eng=None):
                # split into 4 sub-DMAs so the transfer spreads across DGE
                # queues (more SDMA engines in parallel)
                xc = xp.tile([128, 8, NTW], F16, tag="xc", name=f"xc{nt}_{c}")
                e = eng or nc.sync
                for q in range(4):
                    e.dma_start(
                        out=xc[:, 2 * q : 2 * q + 2, :],
                        in_=xt[:, nt, c, 2 * q : 2 * q + 2, :],
                    )
                xc_sb[(nt, c)] = xc

            # ---- small constants (gpsimd queue; keeps sync free for bulk) ----
            ones = smallp.tile([128, 1], F32R)
            nc.gpsimd.dma_start(out=ones, in_=on1[:, :])
            dgt_sb = smallp.tile([128, NSLOT, BLK], F32)
            nc.gpsimd.dma_start(out=dgt_sb, in_=dgt[:, :, :])
            dgr_sb = smallp.tile([128, NSLOT, BLK], F32)
            nc.gpsimd.dma_start(out=dgr_sb, in_=dgr[:, :, :])
            b8_sb = smallp.tile([128, NSLOT], F32)
            nc.gpsimd.dma_start(out=b8_sb, in_=b8[:, :])
            ls8_sb = smallp.tile([128, NSLOT], F32)
            nc.gpsimd.dma_start(out=ls8_sb, in_=ls8[:, :])

            # compact exp of diagonal blocks: a_cmp[p, s, a] = exp(dgt[p, s, a])
            # in one ACT op, then scatter the 32x32 sub-blocks onto the
            # block-diagonal of a_exp with 4 SBUF->SBUF DMAs (zero elsewhere).
            a_cmp = smallp.tile([128, NSLOT, BLK], F16)
            nc.scalar.activation(
                a_cmp.rearrange("p s a -> p (s a)"),
                dgt_sb.rearrange("p s a -> p (s a)"),
                AF.Exp,
            )
            zmem = smallp.tile([128, NSLOT * 128], F32)
            nc.vector.memset(zmem, 0.0)
            a_exp = smallp.tile([128, NSLOT, 128], F16)
            nc.vector.tensor_copy(a_exp.rearrange("p s f -> p (s f)"), zmem)
            for j in range(4):
                pr = slice(32 * j, 32 * j + 32)
                nc.gpsimd.dma_start(out=a_exp[pr, :, 32 * j : 32 * j + 32], in_=a_cmp[pr, :, :])

            w_chunk = []

            def w_tile(kt, s):
                t = T_IDX[(kt, s)]
                c = kt // 4
                t0, _ = _wc_range(c)
                return w_chunk[c][:, (t - t0) * 128 : (t - t0 + 1) * 128]

            def emit_matmuls(s, nt, out_sb, copy_raw):
                """Accumulate slot s, n-tile nt into PSUM, then copy to out_sb.

                copy_raw=True: plain PSUM->SBUF copy (g not available yet);
                the g*psum+bias affine is applied later in place.
                copy_raw=False: fused g*psum+bias copyback.
                """
                xms = xmp.tile([128, NTW], F16, tag="xm", name=f"xm{nt}_{s}")
                nc.gpsimd.dma_start(out=xms, in_=xm[:, nt, s, :])
                ps_y = pyp.tile([128, NTW], F32, tag="psy")
                for kt in range(4 * (s + 1)):
                    nc.tensor.matmul(
                        ps_y,
                        w_tile(kt, s),
                        xc_sb[(nt, kt // 8)][:, kt % 8, :],
                        start=(kt == 0),
                        stop=False,
                    )
                # diag-block exp contribution (block-diagonal lhsT, full K)
                nc.tensor.matmul(
                    ps_y, a_exp[:, s, :], xms, start=False, stop=True
                )
                if copy_raw:
                    nc.scalar.copy(out_sb, ps_y)
                else:
                    nc.scalar.activation(
                        out_sb, ps_y, AF.Identity,
                        bias=b8_sb[:, s : s + 1], scale=g_sb[:, s : s + 1],
                    )

            g_sb = smallp.tile([128, NSLOT], F32)
            ljb = smallp.tile([128, NSLOT], F32)
            # staging for nt=0 results (copied raw before g is known)
            stage = [
                smallp.tile([128, NTW], F32, tag=f"st{s}", name=f"stage{s}")
                for s in range(NSLOT)
            ]

            # ---- interleaved: W chunk DMA -> norm jobs -> main slot (nt=0) ----
            # DMA triggers are emitted in consumption order (wc0, xc0, wc1,
            # wc2, xc1, ...) so early slots' data arrives first; main slot
            # emission lags the chunk loop by 2 so the PE never parks on the
            # a_exp-dependent tail matmul while W/x are still streaming.
            with (
                tc.tile_pool(name="psn", bufs=1, space="PSUM") as pnp,
                tc.tile_pool(name="sqp", bufs=2) as sqp,
            ):
                psn = pnp.tile([1, 1024], F32)

                # W chunk tiles created up front; chunks 0-3 DMA'd immediately
                # (first wave), chunks 4-7 triggered from the ACT engine after
                # early slots' copybacks so DMA issue paces with compute.
                for c in range(NWC):
                    t0c, t1c = _wc_range(c)
                    wc = wp.tile([128, (t1c - t0c) * 128], F16, tag=f"wc{c}", name=f"wct{c}")
                    w_chunk.append(wc)

                def trigger_wc(c, eng):
                    t0c, t1c = _wc_range(c)
                    # split in 2 sub-DMAs for queue spread
                    mid = (t0c + t1c) // 2
                    eng.dma_start(
                        out=w_chunk[c][:, : (mid - t0c) * 128],
                        in_=ws[:, t0c * 128 : mid * 128],
                    )
                    eng.dma_start(
                        out=w_chunk[c][:, (mid - t0c) * 128 :],
                        in_=ws[:, mid * 128 : t1c * 128],
                    )

                # everything up front, consumption-ordered; the DGE round-robin
                # shares bandwidth, order mostly sets start order
                trigger_wc(0, nc.sync)
                emit_xc(0, 0)
                trigger_wc(1, nc.sync)
                emit_xc(0, 1)
                trigger_wc(2, nc.sync)
                trigger_wc(3, nc.sync)
                emit_xc(0, 2)
                trigger_wc(4, nc.sync)
                trigger_wc(5, nc.sync)
                emit_xc(0, 3)
                trigger_wc(6, nc.sync)
                trigger_wc(7, nc.sync)
                for c in range(NXC):  # nt=1 x; all 8 chunks fit in SBUF at fp16
                    emit_xc(1, c)

                for c in range(NWC):
                    c0 = _wc_range(c)[0]
                    for kt in range(4 * c, 4 * c + 4):
                        t0, t1 = _kt_range(kt)
                        nact = t1 - t0
                        w_slice = w_chunk[c][:, (t0 - c0) * 128 : (t1 - c0) * 128]
                        smin = c * 128
                        off = 0
                        while off < nact * 128:
                            n = min(512, nact * 128 - off)
                            sq = sqp.tile([128, 512], F32R, tag="sq")
                            nc.vector.tensor_mul(
                                sq[:, :n],
                                w_slice[:, off : off + n],
                                w_slice[:, off : off + n],
                            )
                            nc.tensor.matmul(
                                psn[0:1, smin + off : smin + off + n],
                                ones,
                                sq[:, :n],
                                start=(kt == 0),
                                stop=False,
                            )
                            off += n
                    # lag main-slot emission 2 chunks behind the DMA stream
                    if c >= 2:
                        emit_matmuls(c - 2, 0, stage[c - 2], copy_raw=True)
                for s in range(NSLOT - 2, NSLOT):
                    emit_matmuls(s, 0, stage[s], copy_raw=True)
                # diag-block contribution to the norms (block-diagonal squares)
                ae_flat = a_exp.rearrange("p s f -> p (s f)")
                for half in range(2):
                    a_sq = sqp.tile([128, 512], F32R, tag="sq", name=f"asq{half}")
                    nc.vector.tensor_mul(
                        a_sq, ae_flat[:, half * 512 : (half + 1) * 512],
                        ae_flat[:, half * 512 : (half + 1) * 512],
                    )
                    nc.tensor.matmul(
                        psn[0:1, half * 512 : (half + 1) * 512], ones, a_sq,
                        start=False, stop=(half == 1),
                    )
                s_row = smallp.tile([1, 1024], F32)
                nc.scalar.copy(s_row, psn[0:1, :])

            # s roundtrip through DRAM to get per-partition layout [128, 8]
            nc.gpsimd.dma_start(out=srt[:].rearrange("(a n) -> a n", a=1), in_=s_row[0:1, :])
            s_col = smallp.tile([128, NSLOT], F32)
            nc.gpsimd.dma_start(out=s_col, in_=srt[:].rearrange("(s p) -> p s", p=128))

            # g = exp(ls - 0.5 ln s);  ljb = ls - 0.5 ln s
            t_ln = smallp.tile([128, NSLOT], F32)
            nc.scalar.activation(t_ln, s_col, AF.Ln)
            for s in range(NSLOT):
                nc.scalar.activation(
                    ljb[:, s : s + 1], t_ln[:, s : s + 1], AF.Identity,
                    bias=ls8_sb[:, s : s + 1], scale=-0.5,
                )
            nc.scalar.activation(g_sb, ljb, AF.Exp)

            # log-jacobian: lj = W_diag + (ls - 0.5 ln s)
            lj_sb = smallp.tile([128, NSLOT, BLK], F32)
            for s in range(NSLOT):
                nc.scalar.activation(
                    lj_sb[:, s, :], dgr_sb[:, s, :], AF.Identity,
                    bias=ljb[:, s : s + 1], scale=1.0,
                )
            nc.gpsimd.dma_start(out=lj[:, :, :], in_=lj_sb)

            # nt=0: apply y = g*acc + bias in place on DVE (ACT is busy with
            # nt=1 copybacks in this window), store via the idle sync queue
            for s in range(NSLOT):
                nc.vector.tensor_scalar(
                    stage[s], stage[s],
                    g_sb[:, s : s + 1], b8_sb[:, s : s + 1],
                    mybir.AluOpType.mult, mybir.AluOpType.add,
                )
                nc.sync.dma_start(
                    out=yt[s * 128 : (s + 1) * 128, 0:NTW], in_=stage[s]
                )

            # ---- second batch half (nt=1; x already resident) ----
            for s in range(NSLOT):
                y_sb = yp.tile([128, NTW], F32, tag="y")
                emit_matmuls(s, 1, y_sb, copy_raw=False)
                nc.sync.dma_start(
                    out=yt[s * 128 : (s + 1) * 128, NTW : 2 * NTW], in_=y_sb
                )
    nc.finalize()
    return nc


def _pack_inputs(x, W, bias, W_log_scale):
    """Host-side data layout: slice/transpose/zero-select only (no math)."""
    f32 = np.float32
    WT = np.ascontiguousarray(W.T.astype(f32, copy=False))
    xT = np.ascontiguousarray(x.T.astype(f32, copy=False))
    bias = bias.astype(f32, copy=False)
    ls = W_log_scale.astype(f32, copy=False)

    grp = {}
    for g in range(NGRP):
        ws_g = np.zeros((128, NSTREAM, 128), np.float16)
        for t, (kt, s) in enumerate(STREAM):
            mg = 4 * s + g
            if kt < mg:
                ws_g[:, t, :] = WT[kt * 128 : (kt + 1) * 128, mg * 128 : (mg + 1) * 128]
            elif kt == mg:
                blk = WT[kt * 128 : (kt + 1) * 128, mg * 128 : (mg + 1) * 128].copy()
                for jp in range(4):  # keep only sub-blocks strictly above the diagonal
                    blk[32 * jp : 32 * jp + 32, : 32 * (jp + 1)] = 0.0
                ws_g[:, t, :] = blk
            # kt > mg: zero pad
        dgt = np.empty((128, NSLOT, BLK), f32)
        dgr = np.empty((128, NSLOT, BLK), f32)
        for s in range(NSLOT):
            mg = 4 * s + g
            blk = W[mg * 128 : (mg + 1) * 128, mg * 128 : (mg + 1) * 128]
            for j in range(4):
                sub = blk[32 * j : 32 * j + 32, 32 * j : 32 * j + 32]  # [a, b]
                dgt[32 * j : 32 * j + 32, s, :] = sub.T
                dgr[32 * j : 32 * j + 32, s, :] = sub
        b8 = np.stack(
            [bias[(4 * s + g) * 128 : (4 * s + g + 1) * 128] for s in range(NSLOT)], 1
        )
        ls8 = np.stack(
            [ls[(4 * s + g) * 128 : (4 * s + g + 1) * 128, 0] for s in range(NSLOT)], 1
        )
        grp[g] = dict(
            ws=np.ascontiguousarray(ws_g.reshape(128, NSTREAM * 128)),
            dgt=dgt, dgr=dgr, b8=np.ascontiguousarray(b8), ls8=np.ascontiguousarray(ls8),
        )

    # x in per-core SBUF layout: xt[p, nt, c, j, n] = xT[(8c+j)*128+p, h*1024+nt*512+n]
    xt_h = {}
    xm_h = {}
    for h in range(2):
        xs = xT[:, h * 1024 : (h + 1) * 1024]               # [4096, 1024]
        v = xs.reshape(KT, 128, NT, NTW)                     # [kt, p, nt, n]
        v = v.transpose(1, 2, 0, 3)                          # [p, nt, kt, n]
        xt_h[h] = np.ascontiguousarray(v.reshape(128, NT, NXC, 8, NTW).astype(np.float16))
    in_maps = []
    for c in range(8):
        g, h = c // 2, c % 2
        xm_c = np.empty((128, NT, NSLOT, NTW), np.float16)
        for s in range(NSLOT):
            mg = 4 * s + g
            for nt in range(NT):
                xm_c[:, nt, s, :] = xT[
                    mg * 128 : (mg + 1) * 128,
                    h * 1024 + nt * NTW : h * 1024 + (nt + 1) * NTW,
                ]
        in_maps.append(
            {
                **grp[g],
                "xt": xt_h[h],
                "xm": xm_c,
                "on1": np.ones((128, 1), f32),
            }
        )
    return in_maps


_PROGRAM = None


def kernel(x, W, bias, W_log_scale, b_diag_mask=None, b_tril_mask=None, **_ignored):
    """Full inputs in, full outputs out. Returns (y, log_jac) like the reference.

    The diag/tril masks are the fixed block-kronecker patterns of the module
    (block-diagonal + strict block-lower-triangular); the kernel exploits that
    structure directly, so the mask tensors themselves are not transferred.
    """
    from concourse.bass_utils import run_bass_kernel_spmd

    global _PROGRAM
    if _PROGRAM is None:
        _PROGRAM = _build_program()
    nc = _PROGRAM

    in_maps = _pack_inputs(
        np.asarray(x), np.asarray(W), np.asarray(bias), np.asarray(W_log_scale)
    )
    res = run_bass_kernel_spmd(nc, in_maps, list(range(8)))

    y = np.empty((BATCH, DIM), np.float32)
    lj_full = np.empty((NB, BLK, BLK), np.float32)
    for c, r in enumerate(res.results):
        g, h = c // 2, c % 2
        ytc = r["yt"]
        for s in range(NSLOT):
            mg = 4 * s + g
            y[h * 1024 : (h + 1) * 1024, mg * 128 : (mg + 1) * 128] = (
                ytc[s * 128 : (s + 1) * 128, :].T
            )
        if h == 0:
            ljc = r["lj"]  # [128, 8, 32]
            for s in range(NSLOT):
                mg = 4 * s + g
                for j in range(4):
                    lj_full[4 * mg + j] = ljc[32 * j : 32 * j + 32, s, :]
    return (y, lj_full)


# revision 57
# speedup vs baseline: 1.7672x; 1.0097x over previous
"""Trainium2 Bass kernel for BlockAutoregressiveLinear (n_blocks=128, B=32,
in=out=4096, batch=2048), 8 NeuronCores.

Math (see reference):
    Wm = exp(W)*diag_mask + W*tril_mask          (block-diag exp, strict block-lower copy)
    s[o] = sum_i Wm[o,i]^2
    g[o] = exp(W_log_scale[o] - 0.5*ln s[o])
    y = x @ (g[:,None]*Wm).T + bias
    log_jac[k,a,b] = W[k*32+a, k*32+b] + W_log_scale[o] - 0.5*ln s[o]

Sharding: 4 column groups x 2 batch halves. Core c=(2g+h) owns 8 output
m-tiles (128 rows each) mg = 4s+g for slot s=0..7 and batch half h.
W is block-lower-triangular, so m-tile mg only needs k-tiles 0..mg. For an
SPMD-uniform instruction stream, slot s is padded to K_s = 4(s+1) k-tile
matmuls (pad tiles are host-packed zeros); every core executes the identical
program and all per-core variation lives in the input data.

The diagonal-block exp contribution is a separate per-slot matmul (a_exp
tile built on device with ACT Exp) against a host-packed copy of the
corresponding x rows (xm), so the stream tiles are pure copies of W.T
(masked on host by zeroing = data selection; all arithmetic is on device).

Main matmuls run in fp16 (the weight-normalization cancels most of the
W-quantization error; measured end-to-end rel err ~3e-4, same class as
fp32r) with N=512 moving tiles; the row-norm reduction runs in f32r.
The weight scaling by g and the bias add are fused into the PSUM->SBUF
copyback on the ACT engine (per-partition scale/bias).
"""

import sys

sys.path.insert(0, "/opt/trn_rl_repo")

import numpy as np

NB = 128          # autoregressive blocks
BLK = 32          # block size
DIM = 4096
BATCH = 2048
NSLOT = 8         # m-tile slots per core
NGRP = 4          # column groups
NT = 2            # n-tiles per batch half
NTW = 512         # n-tile width
KT = 32           # k tiles
NWC = 8           # W stream chunks (4 k-tiles each)
NXC = 4           # x chunks per n-tile (8 k-tiles each)

# k-major stream of (kt, slot) jobs; slot s is active for kt < 4(s+1)
STREAM = [(kt, s) for kt in range(KT) for s in range(NSLOT) if kt < 4 * (s + 1)]
T_IDX = {j: t for t, j in enumerate(STREAM)}
NSTREAM = len(STREAM)  # 144
# row-norm slice: per slot s a [128, 32 + K_s*128] region (diag row part first,
# then the strict-lower row, zero-padded to the uniform class width)
WR_W = [32 + 4 * (s + 1) * 128 for s in range(NSLOT)]
WR_OFF = [sum(WR_W[:s]) for s in range(NSLOT)]
WR_TOT = sum(WR_W)  # 4864


def _kt_range(kt):
    """[t0, t1) tile-index range of stream tiles belonging to k-tile kt."""
    t0 = T_IDX[(kt, kt // 4)]
    return t0, t0 + NSLOT - kt // 4


def _wc_range(c):
    """[t0, t1) tile-index range of W chunk c (k-tiles 4c..4c+3)."""
    return _kt_range(4 * c)[0], _kt_range(4 * c + 3)[1]


def _build_program():
    import concourse.tile as tile
    from concourse import bacc, mybir

    F32 = mybir.dt.float32
    F32R = mybir.dt.float32r
    F16 = mybir.dt.float16
    AF = mybir.ActivationFunctionType

    nc = bacc.Bacc("TRN2", target_bir_lowering=False, debug=False, num_devices=8)

    ws = nc.dram_tensor("ws", [128, NSTREAM * 128], F16, kind="ExternalInput")
    # x, host-packed per-core in SBUF layout: [p, nt, xchunk, ktile-in-chunk, n]
    xt = nc.dram_tensor("xt", [128, NT, NXC, 8, NTW], F16, kind="ExternalInput")
    xm = nc.dram_tensor("xm", [128, NT, NSLOT, NTW], F16, kind="ExternalInput")
    dgt = nc.dram_tensor("dgt", [128, NSLOT, BLK], F32, kind="ExternalInput")
    dgr = nc.dram_tensor("dgr", [128, NSLOT, BLK], F32, kind="ExternalInput")
    b8 = nc.dram_tensor("b8", [128, NSLOT], F32, kind="ExternalInput")
    ls8 = nc.dram_tensor("ls8", [128, NSLOT], F32, kind="ExternalInput")
    on1 = nc.dram_tensor("on1", [128, 1], F32R, kind="ExternalInput")
    wr = nc.dram_tensor("wr", [128, WR_TOT], F16, kind="ExternalInput")
    yt = nc.dram_tensor("yt", [1024, 1024], F32, kind="ExternalOutput")
    lj = nc.dram_tensor("lj", [128, NSLOT, BLK], F32, kind="ExternalOutput")
    srt = nc.dram_tensor("srt", [1024], F32)  # s roundtrip scratch

    with tile.TileContext(nc) as tc:
        with (
            tc.tile_pool(name="wp", bufs=1) as wp,
            tc.tile_pool(name="xp", bufs=8) as xp,
            tc.tile_pool(name="xmp", bufs=4) as xmp,
            tc.tile_pool(name="yp", bufs=4) as yp,
            tc.tile_pool(name="smallp", bufs=1) as smallp,
            tc.tile_pool(name="psy", bufs=6, space="PSUM") as pyp,
        ):
            xc_sb = {}

            def emit_xc(nt, c, eng=None):
                # split into 4 sub-DMAs so the transfer spreads across DGE
                # queues (more SDMA engines in parallel)
                xc = xp.tile([128, 8, NTW], F16, tag="xc", name=f"xc{nt}_{c}")
                e = eng or nc.sync
                for q in range(4):
                    e.dma_start(
                        out=xc[:, 2 * q : 2 * q + 2, :],
                        in_=xt[:, nt, c, 2 * q : 2 * q + 2, :],
                    )
                xc_sb[(nt, c)] = xc

            # ---- small constants (gpsimd queue; keeps sync free for bulk) ----
            ones = smallp.tile([128, 1], F32R)
            nc.gpsimd.dma_start(out=ones, in_=on1[:, :])
            dgt_sb = smallp.tile([128, NSLOT, BLK], F32)
            nc.gpsimd.dma_start(out=dgt_sb, in_=dgt[:, :, :])
            dgr_sb = smallp.tile([128, NSLOT, BLK], F32)
            nc.gpsimd.dma_start(out=dgr_sb, in_=dgr[:, :, :])
            b8_sb = smallp.tile([128, NSLOT], F32)
            nc.gpsimd.dma_start(out=b8_sb, in_=b8[:, :])
            ls8_sb = smallp.tile([128, NSLOT], F32)
            nc.gpsimd.dma_start(out=ls8_sb, in_=ls8[:, :])

            # compact exp of diagonal blocks: a_cmp[p, s, a] = exp(dgt[p, s, a])
            # in one ACT op, then scatter the 32x32 sub-blocks onto the
            # block-diagonal of a_exp with 4 SBUF->SBUF DMAs (zero elsewhere).
            a_cmp = smallp.tile([128, NSLOT, BLK], F16)
            nc.scalar.activation(
                a_cmp.rearrange("p s a -> p (s a)"),
                dgt_sb.rearrange("p s a -> p (s a)"),
                AF.Exp,
            )
            zmem = smallp.tile([128, NSLOT * 128], F32)
            nc.vector.memset(zmem, 0.0)
            a_exp = smallp.tile([128, NSLOT, 128], F16)
            nc.vector.tensor_copy(a_exp.rearrange("p s f -> p (s f)"), zmem)
            for j in range(4):
                pr = slice(32 * j, 32 * j + 32)
                nc.gpsimd.dma_start(out=a_exp[pr, :, 32 * j : 32 * j + 32], in_=a_cmp[pr, :, :])

            w_chunk = []

            def w_tile(kt, s):
                t = T_IDX[(kt, s)]
                c = kt // 4
                t0, _ = _wc_range(c)
                return w_chunk[c][:, (t - t0) * 128 : (t - t0 + 1) * 128]

            def emit_matmuls(s, nt, out_sb, copy_raw):
                """Accumulate slot s, n-tile nt into PSUM, then copy to out_sb.

                copy_raw=True: plain PSUM->SBUF copy (g not available yet);
                the g*psum+bias affine is applied later in place.
                copy_raw=False: fused g*psum+bias copyback.
                """
                xms = xmp.tile([128, NTW], F16, tag="xm", name=f"xm{nt}_{s}")
                nc.gpsimd.dma_start(out=xms, in_=xm[:, nt, s, :])
                ps_y = pyp.tile([128, NTW], F32, tag="psy")
                for kt in range(4 * (s + 1)):
                    nc.tensor.matmul(
                        ps_y,
                        w_tile(kt, s),
                        xc_sb[(nt, kt // 8)][:, kt % 8, :],
                        start=(kt == 0),
                        stop=False,
                    )
                # diag-block exp contribution (block-diagonal lhsT, full K)
                nc.tensor.matmul(
                    ps_y, a_exp[:, s, :], xms, start=False, stop=True
                )
                if copy_raw:
                    nc.scalar.copy(out_sb, ps_y)
                else:
                    nc.scalar.activation(
                        out_sb, ps_y, AF.Identity,
                        bias=b8_sb[:, s : s + 1], scale=g_sb[:, s : s + 1],
                    )

            g_sb = smallp.tile([128, NSLOT], F32)
            ljb = smallp.tile([128, NSLOT], F32)
            # staging for nt=0 results (copied raw before g is known)
            wr_sb = smallp.tile([128, WR_TOT], F16)
            wrsq_dump = smallp.tile([128, max(WR_W)], F16)
            stage = [
                smallp.tile([128, NTW], F32, tag=f"st{s}", name=f"stage{s}")
                for s in range(NSLOT)
            ]

            # ---- interleaved: W chunk DMA -> norm jobs -> main slot (nt=0) ----
            # DMA triggers are emitted in consumption order (wc0, xc0, wc1,
            # wc2, xc1, ...) so early slots' data arrives first; main slot
            # emission lags the chunk loop by 2 so the PE never parks on the
            # a_exp-dependent tail matmul while W/x are still streaming.
            if True:
                # W chunk tiles created up front; chunks 0-3 DMA'd immediately
                # (first wave), chunks 4-7 triggered from the ACT engine after
                # early slots' copybacks so DMA issue paces with compute.
                for c in range(NWC):
                    t0c, t1c = _wc_range(c)
                    wc = wp.tile([128, (t1c - t0c) * 128], F16, tag=f"wc{c}", name=f"wct{c}")
                    w_chunk.append(wc)

                def trigger_wc(c, eng):
                    t0c, t1c = _wc_range(c)
                    # split in 2 sub-DMAs for queue spread
                    mid = (t0c + t1c) // 2
                    eng.dma_start(
                        out=w_chunk[c][:, : (mid - t0c) * 128],
                        in_=ws[:, t0c * 128 : mid * 128],
                    )
                    eng.dma_start(
                        out=w_chunk[c][:, (mid - t0c) * 128 :],
                        in_=ws[:, mid * 128 : t1c * 128],
                    )

                # everything up front, consumption-ordered; the DGE round-robin
                # shares bandwidth, order mostly sets start order
                trigger_wc(0, nc.sync)
                emit_xc(0, 0)
                trigger_wc(1, nc.sync)
                emit_xc(0, 1)
                trigger_wc(2, nc.sync)
                trigger_wc(3, nc.sync)
                emit_xc(0, 2)
                trigger_wc(4, nc.sync)
                trigger_wc(5, nc.sync)
                emit_xc(0, 3)
                trigger_wc(6, nc.sync)
                trigger_wc(7, nc.sync)
                half = WR_TOT // 2
                nc.sync.dma_start(out=wr_sb[:, :half], in_=wr[:, :half])
                nc.sync.dma_start(out=wr_sb[:, half:], in_=wr[:, half:])
                for c in range(NXC):  # nt=1 x; all 8 chunks fit in SBUF at fp16
                    emit_xc(1, c)

                for c in range(NWC):
                    # lag main-slot emission 2 chunks behind the DMA stream
                    if c >= 2:
                        emit_matmuls(c - 2, 0, stage[c - 2], copy_raw=True)
                for s in range(NSLOT - 2, NSLOT):
                    emit_matmuls(s, 0, stage[s], copy_raw=True)
            # ---- row norms via ACT Square+accum over the row-major slice ----
            s_col = smallp.tile([128, NSLOT], F32)
            for s in range(NSLOT):
                nc.scalar.activation(
                    wr_sb[:, WR_OFF[s] : WR_OFF[s] + 32],
                    wr_sb[:, WR_OFF[s] : WR_OFF[s] + 32],
                    AF.Exp,
                )
                nc.scalar.activation(
                    wrsq_dump[:, : WR_W[s]],
                    wr_sb[:, WR_OFF[s] : WR_OFF[s] + WR_W[s]],
                    AF.Square,
                    accum_out=s_col[:, s : s + 1],
                )

            # g = exp(ls - 0.5 ln s);  ljb = ls - 0.5 ln s
            t_ln = smallp.tile([128, NSLOT], F32)
            nc.scalar.activation(t_ln, s_col, AF.Ln)
            for s in range(NSLOT):
                nc.scalar.activation(
                    ljb[:, s : s + 1], t_ln[:, s : s + 1], AF.Identity,
                    bias=ls8_sb[:, s : s + 1], scale=-0.5,
                )
            nc.scalar.activation(g_sb, ljb, AF.Exp)

            # log-jacobian: lj = W_diag + (ls - 0.5 ln s)
            lj_sb = smallp.tile([128, NSLOT, BLK], F32)
            for s in range(NSLOT):
                nc.scalar.activation(
                    lj_sb[:, s, :], dgr_sb[:, s, :], AF.Identity,
                    bias=ljb[:, s : s + 1], scale=1.0,
                )
            nc.gpsimd.dma_start(out=lj[:, :, :], in_=lj_sb)

            # nt=0: apply y = g*acc + bias in place on DVE (ACT is busy with
            # nt=1 copybacks in this window), store via the idle sync queue
            for s in range(NSLOT):
                nc.vector.tensor_scalar(
                    stage[s], stage[s],
                    g_sb[:, s : s + 1], b8_sb[:, s : s + 1],
                    mybir.AluOpType.mult, mybir.AluOpType.add,
                )
                nc.sync.dma_start(
                    out=yt[s * 128 : (s + 1) * 128, 0:NTW], in_=stage[s]
                )

            # ---- second batch half (nt=1; x already resident) ----
            for s in range(NSLOT):
                y_sb = yp.tile([128, NTW], F32, tag="y")
                emit_matmuls(s, 1, y_sb, copy_raw=False)
                nc.sync.dma_start(
                    out=yt[s * 128 : (s + 1) * 128, NTW : 2 * NTW], in_=y_sb
                )
    nc.finalize()
    return nc


def _pack_inputs(x, W, bias, W_log_scale):
    """Host-side data layout: slice/transpose/zero-select only (no math)."""
    f32 = np.float32
    WT = np.ascontiguousarray(W.T.astype(f32, copy=False))
    xT = np.ascontiguousarray(x.T.astype(f32, copy=False))
    bias = bias.astype(f32, copy=False)
    ls = W_log_scale.astype(f32, copy=False)

    grp = {}
    for g in range(NGRP):
        ws_g = np.zeros((128, NSTREAM, 128), np.float16)
        for t, (kt, s) in enumerate(STREAM):
            mg = 4 * s + g
            if kt < mg:
                ws_g[:, t, :] = WT[kt * 128 : (kt + 1) * 128, mg * 128 : (mg + 1) * 128]
            elif kt == mg:
                blk = WT[kt * 128 : (kt + 1) * 128, mg * 128 : (mg + 1) * 128].copy()
                for jp in range(4):  # keep only sub-blocks strictly above the diagonal
                    blk[32 * jp : 32 * jp + 32, : 32 * (jp + 1)] = 0.0
                ws_g[:, t, :] = blk
            # kt > mg: zero pad
        dgt = np.empty((128, NSLOT, BLK), f32)
        dgr = np.empty((128, NSLOT, BLK), f32)
        for s in range(NSLOT):
            mg = 4 * s + g
            blk = W[mg * 128 : (mg + 1) * 128, mg * 128 : (mg + 1) * 128]
            for j in range(4):
                sub = blk[32 * j : 32 * j + 32, 32 * j : 32 * j + 32]  # [a, b]
                dgt[32 * j : 32 * j + 32, s, :] = sub.T
                dgr[32 * j : 32 * j + 32, s, :] = sub
        b8 = np.stack(
            [bias[(4 * s + g) * 128 : (4 * s + g + 1) * 128] for s in range(NSLOT)], 1
        )
        ls8 = np.stack(
            [ls[(4 * s + g) * 128 : (4 * s + g + 1) * 128, 0] for s in range(NSLOT)], 1
        )
        wrg = np.zeros((128, WR_TOT), np.float16)
        for s in range(NSLOT):
            mg = 4 * s + g
            off = WR_OFF[s]
            for p in range(128):
                o = mg * 128 + p
                k = mg * 4 + p // 32
                wrg[p, off : off + 32] = W[o, k * 32 : k * 32 + 32]
                wrg[p, off + 32 : off + 32 + k * 32] = W[o, : k * 32]
        grp[g] = dict(
            ws=np.ascontiguousarray(ws_g.reshape(128, NSTREAM * 128)),
            wr=wrg,
            dgt=dgt, dgr=dgr, b8=np.ascontiguousarray(b8), ls8=np.ascontiguousarray(ls8),
        )

    # x in per-core SBUF layout: xt[p, nt, c, j, n] = xT[(8c+j)*128+p, h*1024+nt*512+n]
    xt_h = {}
    xm_h = {}
    for h in range(2):
        xs = xT[:, h * 1024 : (h + 1) * 1024]               # [4096, 1024]
        v = xs.reshape(KT, 128, NT, NTW)                     # [kt, p, nt, n]
        v = v.transpose(1, 2, 0, 3)                          # [p, nt, kt, n]
        xt_h[h] = np.ascontiguousarray(v.reshape(128, NT, NXC, 8, NTW).astype(np.float16))
    in_maps = []
    for c in range(8):
        g, h = c // 2, c % 2
        xm_c = np.empty((128, NT, NSLOT, NTW), np.float16)
        for s in range(NSLOT):
            mg = 4 * s + g
            for nt in range(NT):
                xm_c[:, nt, s, :] = xT[
                    mg * 128 : (mg + 1) * 128,
                    h * 1024 + nt * NTW : h * 1024 + (nt + 1) * NTW,
                ]
        in_maps.append(
            {
                **grp[g],
                "xt": xt_h[h],
                "xm": xm_c,
                "on1": np.ones((128, 1), f32),
            }
        )
    return in_maps


_PROGRAM = None


def kernel(x, W, bias, W_log_scale, b_diag_mask=None, b_tril_mask=None, **_ignored):
    """Full inputs in, full outputs out. Returns (y, log_jac) like the reference.

    The diag/tril masks are the fixed block-kronecker patterns of the module
    (block-diagonal + strict block-lower-triangular); the kernel exploits that
    structure directly, so the mask tensors themselves are not transferred.
    """
    from concourse.bass_utils import run_bass_kernel_spmd

    global _PROGRAM
    if _PROGRAM is None:
        _PROGRAM = _build_program()
    nc = _PROGRAM

    in_maps = _pack_inputs(
        np.asarray(x), np.asarray(W), np.asarray(bias), np.asarray(W_log_scale)
    )
    res = run_bass_kernel_spmd(nc, in_maps, list(range(8)))

    y = np.empty((BATCH, DIM), np.float32)
    lj_full = np.empty((NB, BLK, BLK), np.float32)
    for c, r in enumerate(res.results):
        g, h = c // 2, c % 2
        ytc = r["yt"]
        for s in range(NSLOT):
            mg = 4 * s + g
            y[h * 1024 : (h + 1) * 1024, mg * 128 : (mg + 1) * 128] = (
                ytc[s * 128 : (s + 1) * 128, :].T
            )
        if h == 0:
            ljc = r["lj"]  # [128, 8, 32]
            for s in range(NSLOT):
                mg = 4 * s + g
                for j in range(4):
                    lj_full[4 * mg + j] = ljc[32 * j : 32 * j + 32, s, :]
    return (y, lj_full)
